# revision 13
# baseline (speedup 1.0000x reference)
"""PointNet-style encoder (conv1x1 stack + ragged segment-max) on 8 Trainium2 cores.

Strategy (v2.1 — tail-tile packing)
-----------------------------------
* BN folded into conv weights host-side; every layer is matmul+bias+ReLU.
* Feature-major on device: activations live as [C, points] tiles; points stream
  through the PE as the matmul free dimension in 512-point macro-tiles.
* Segments are point-balanced across the 8 cores (whole segments per core, so
  the two segment-maxes stay core-local). Per core the layout is:
    - T_TAIL fixed "tail" tiles (array idx 0..T_TAIL-1) holding every
      segment's sub-512 remainder, packed as 64-col-aligned per-slot chunks;
    - P_PURE "pure" tiles (idx T_TAIL..) each belonging to one segment.
  This wastes ~1 tile/core instead of the ~3 of per-segment padding.
* Phase A (L1+L2) runs tail tiles first, then pures in slot order; per-tile
  f2 maxes (Mx2) land in a combined column buffer: 8 per-64-col group maxes
  per tail tile, 1 per pure tile. Per-slot g = masked max over that buffer
  (gmask input zeros other slots; buffer memset 0 + f2>=0 keeps unwritten
  columns neutral).
* Mid-network unpool via concat identity: concat(f2, g)@W3 = f2@W3a + g@W3b.
  Per slot Us = W3b^T g + b3 (tiny 1-col matmuls). Pure tiles read a per-tile
  bias column Ub[:,m,j], built by one masked broadcast pass per slot (umask2
  input). Tail tiles get the g-term exactly via an extra accumulated matmul
  per m-chunk: lhsT = UsT (computed on-PE as G2^T@W3b, [S,512]), rhs =
  one-hot slot matrix O [S, cols] — per-point unpool without masks.
* Phase B (L3+L4) runs pures first (unlocked per slot as g arrives), tail
  tiles last (they need every slot's UsT). L4 maxes reduce per pure tile and
  per 64-col group for tail tiles (raw, pre-bias); the host applies
  relu(.+b4) and combines columns per segment (exact under max).
* Single interleaved pipeline (phase A runs LEAD tiles ahead of phase B) so
  A's ACT/DVE drains hide under B's PE-bound matmuls; dummy matmuls keep the
  PE HAM clock-gate open through the drain-paced fill phase.
* Matmuls in float16 (fp32 PSUM): 1 cycle/column, ~1e-3 rel err.
"""

import numpy as np

import concourse.bass as bass
import concourse.mybir as mybir
import concourse.tile as tile
from concourse import bacc
from concourse.bass_utils import run_bass_kernel_spmd

N_CORES = 8
PT = 512
GRP = 64  # tail group granularity (cols)
NG = PT // GRP  # groups per tile
EPS = 1e-3  # keras BatchNormalization default epsilon

F32 = mybir.dt.float32
F16 = mybir.dt.float16
AF = mybir.ActivationFunctionType
AXX = mybir.AxisListType.X
AXXY = mybir.AxisListType.XY
ALU_MAX = mybir.AluOpType.max
ALU_ADD = mybir.AluOpType.add

_PROGRAM_CACHE: dict = {}


def _build_program(T_tail: int, P_pure: int, S: int, bounds: tuple, shi_pure: tuple):
    """One SPMD program for all cores.

    bounds[s]: #A-tiles after which slot s's f2 is complete on every core.
    shi_pure[j]: max (over cores) slot id of pure tile j.
    """
    TM = T_tail + P_pure
    NTC = T_tail * NG  # tail group columns
    GW = NTC + P_pure  # combined max-buffer width

    nc = bacc.Bacc("TRN2")

    xT = nc.dram_tensor("xT", [3, TM * PT], F16, kind="ExternalInput")
    gmask = nc.dram_tensor("gmask", [128, S, GW], F32, kind="ExternalInput")
    umask2 = nc.dram_tensor("umask2", [128, S, P_pure], F32, kind="ExternalInput")
    onehot = nc.dram_tensor("onehot", [S, T_tail * PT], F16, kind="ExternalInput")
    w1 = nc.dram_tensor("w1", [3, 128], F16, kind="ExternalInput")
    w2 = nc.dram_tensor("w2", [128, 2, 128], F16, kind="ExternalInput")
    w3a = nc.dram_tensor("w3a", [128, 2, 4, 128], F16, kind="ExternalInput")
    w3b = nc.dram_tensor("w3b", [128, 2, 4, 128], F16, kind="ExternalInput")
    w4 = nc.dram_tensor("w4", [128, 4, 8, 128], F16, kind="ExternalInput")
    b1 = nc.dram_tensor("b1", [128, 1], F32, kind="ExternalInput")
    b2 = nc.dram_tensor("b2", [128, 2], F32, kind="ExternalInput")
    b3 = nc.dram_tensor("b3", [128, 4], F32, kind="ExternalInput")
    mx4 = nc.dram_tensor("mx4", [128, 8, GW], F32, kind="ExternalOutput")

    X_HEAD = min(6, TM)  # tiles in the first (prioritized) x DMA

    with tile.TileContext(nc) as tc:
        with (
            tc.tile_pool(name="const", bufs=1) as constp,
            tc.tile_pool(name="h1p", bufs=3) as h1p,
            tc.tile_pool(name="h3p", bufs=3) as h3p,
            tc.tile_pool(name="tmpp", bufs=4) as tmpp,
            tc.tile_pool(name="psA", bufs=2, space="PSUM") as psA,
            tc.tile_pool(name="psB3", bufs=2, space="PSUM") as psB3,
            tc.tile_pool(name="psB4", bufs=2, space="PSUM") as psB4,
        ):
            # x head first on the sync queue so the first L1 unblocks ASAP
            x_sb = constp.tile([3, TM * PT], F16)
            nc.sync.dma_start(out=x_sb[:, : X_HEAD * PT], in_=xT.ap()[:, : X_HEAD * PT])
            w1_sb = constp.tile([3, 128], F16)
            nc.sync.dma_start(out=w1_sb, in_=w1.ap())
            w2_sb = constp.tile([128, 2, 128], F16)
            nc.sync.dma_start(out=w2_sb, in_=w2.ap())
            b1_sb = constp.tile([128, 1], F32)
            nc.sync.dma_start(out=b1_sb, in_=b1.ap())
            b2_sb = constp.tile([128, 2], F32)
            nc.sync.dma_start(out=b2_sb, in_=b2.ap())
            b3_sb = constp.tile([128, 4], F32)
            nc.sync.dma_start(out=b3_sb, in_=b3.ap())
            nc.sync.dma_start(out=x_sb[:, X_HEAD * PT :], in_=xT.ap()[:, X_HEAD * PT :])
            # phase-B weights + masks on the gpsimd queue, in need order
            w3a_sb = constp.tile([128, 2, 4, 128], F16)
            nc.gpsimd.dma_start(out=w3a_sb, in_=w3a.ap())
            gmask_sb = constp.tile([128, S, GW], F32)
            nc.gpsimd.dma_start(out=gmask_sb, in_=gmask.ap())
            umask2_sb = constp.tile([128, S, P_pure], F32)
            nc.gpsimd.dma_start(out=umask2_sb, in_=umask2.ap())
            w3b_sb = constp.tile([128, 2, 4, 128], F16)
            nc.gpsimd.dma_start(out=w3b_sb, in_=w3b.ap())
            w4_sb = constp.tile([128, 4, 8, 128], F16)
            nc.gpsimd.dma_start(out=w4_sb, in_=w4.ap())
            onehot_sb = constp.tile([S, T_tail * PT], F16)
            nc.gpsimd.dma_start(out=onehot_sb, in_=onehot.ap())

            # f2 storage: tail tiles keep a grouped view for 64-col reduces
            f2_tail = constp.tile([128, T_tail, 2, NG, GRP], F16)
            f2_pure = constp.tile([128, P_pure, 2, PT], F16)
            Mx2_sb = constp.tile([128, 2, GW], F32)
            g_sb = constp.tile([128, 2, S], F32)
            G2s_sb = constp.tile([128, 2, S], F16)
            Us_sb = constp.tile([128, 4, S], F32)
            Ub_sb = constp.tile([128, 4, P_pure], F32)
            UsT_sb = constp.tile([S, 4, 128], F16)
            Mx4_sb = constp.tile([128, 8, GW], F32)

            # zero-init buffers that masked ops may read before fully written
            nc.vector.memset(Mx2_sb, 0.0)
            nc.vector.memset(Us_sb, 0.0)
            nc.vector.memset(G2s_sb, 0.0)

            # HAM warmup: dependency-free matmuls during the DMA prologue
            warm_src = constp.tile([128, PT], F16, name="warm_src")
            nc.vector.memset(warm_src, 0.01)
            warm_out = constp.tile([128, 1], F32, name="warm_out")
            warm_act = constp.tile([128, 8], F32, name="warm_act")
            nc.scalar.activation(out=warm_act, in_=warm_src[:, 0:8], func=AF.Relu)
            ps_dummy = psB4.tile([128, 2, PT], F32, tag="ps4", name="ps_warm")
            for _ in range(4):
                nc.tensor.matmul(
                    ps_dummy[:, 0, :], warm_src[:, 0:128], warm_src[:, :], start=True, stop=True
                )
            nc.vector.tensor_reduce(out=warm_out, in_=ps_dummy[:, 0, 0:8], axis=AXX, op=ALU_MAX)

            def f2v(t):
                return f2_tail[:, t] if t < T_tail else f2_pure[:, t - T_tail]

            deferred_mx2: list = []

            def emit_mx2(t):
                if t < T_tail:
                    nc.vector.tensor_reduce(
                        out=Mx2_sb[:, :, t * NG : (t + 1) * NG], in_=f2_tail[:, t],
                        axis=AXX, op=ALU_MAX,
                    )
                else:
                    j = t - T_tail
                    nc.vector.tensor_reduce(
                        out=Mx2_sb[:, :, NTC + j : NTC + j + 1], in_=f2_pure[:, j],
                        axis=AXX, op=ALU_MAX,
                    )

            def emit_A(t, fill):
                """L1+L2 for array tile t; stores f2 (fp16) + its Mx2 column(s)."""
                tail = t < T_tail
                shp = [128, NG, GRP] if tail else [128, PT]
                ps1 = psA.tile(shp, F32, tag="psa", name=f"ps1_{t}")
                nc.tensor.matmul(
                    ps1[:, :], w1_sb[:, :], x_sb[:, t * PT : (t + 1) * PT],
                    start=True, stop=True,
                )
                h1_sb = h1p.tile(shp, F16, tag="h1", name=f"h1_{t}")
                nc.scalar.activation(out=h1_sb, in_=ps1, func=AF.Relu, bias=b1_sb[:, 0:1])
                for c in range(2):
                    ps2 = psA.tile(shp, F32, tag="psa", name=f"ps2_{t}_{c}")
                    nc.tensor.matmul(ps2[:, :], w2_sb[:, c, :], h1_sb[:, :], start=True, stop=True)
                    # engine split: fill is drain-paced, steady state hides
                    # drains under phase B; Mx2 for pre-bounds[0] tiles must
                    # run inline (slot 0 unlock), later fill tiles defer
                    if fill and t >= bounds[0]:
                        on_dve = True  # both chunks on DVE, no Mx2 inline
                    else:
                        on_dve = c == 1 and fill
                    if not fill:
                        on_dve = False  # steady: ACT takes all three drains
                    if on_dve:
                        nc.vector.tensor_scalar(
                            f2v(t)[:, c], ps2, b2_sb[:, c : c + 1], 0.0, ALU_ADD, ALU_MAX
                        )
                    else:
                        nc.scalar.activation(
                            out=f2v(t)[:, c], in_=ps2, func=AF.Relu, bias=b2_sb[:, c : c + 1]
                        )
                if fill and t >= bounds[0]:
                    deferred_mx2.append(t)
                else:
                    emit_mx2(t)

            def emit_g(s):
                """g[s] = masked max over the Mx2 buffer; then Us[s], Ub pass."""
                for c in range(2):
                    tmp = tmpp.tile([128, GW], F32, tag="tmp", name=f"tmpg_{c}_{s}")
                    nc.vector.tensor_mul(tmp[:, :], Mx2_sb[:, c, :], gmask_sb[:, s, :])
                    nc.vector.tensor_reduce(
                        out=g_sb[:, c, s : s + 1], in_=tmp[:, :], axis=AXX, op=ALU_MAX
                    )
                nc.scalar.copy(G2s_sb[:, :, s], g_sb[:, :, s])
                psu = psA.tile([128, PT], F32, tag="psa", name=f"psu_{s}")
                for m in range(4):
                    nc.tensor.matmul(
                        psu[:, m : m + 1], w3b_sb[:, 0, m, :], G2s_sb[:, 0, s : s + 1],
                        start=True, stop=False,
                    )
                    nc.tensor.matmul(
                        psu[:, m : m + 1], w3b_sb[:, 1, m, :], G2s_sb[:, 1, s : s + 1],
                        start=False, stop=True,
                    )
                for m in range(4):
                    nc.scalar.activation(
                        out=Us_sb[:, m, s : s + 1], in_=psu[:, m : m + 1],
                        func=AF.Identity, bias=b3_sb[:, m : m + 1],
                    )
                # broadcast Us[slot] into the per-tile bias table
                for m in range(4):
                    if s == 0:
                        nc.vector.tensor_scalar_mul(
                            Ub_sb[:, m, :], umask2_sb[:, s, :], Us_sb[:, m, s : s + 1]
                        )
                    else:
                        tmpu = tmpp.tile([128, P_pure], F32, tag="tmpu", name=f"tmpu_{s}_{m}")
                        nc.vector.tensor_scalar_mul(
                            tmpu[:, :], umask2_sb[:, s, :], Us_sb[:, m, s : s + 1]
                        )
                        nc.vector.tensor_add(Ub_sb[:, m, :], Ub_sb[:, m, :], tmpu[:, :])

            def emit_UsT():
                """UsT = (G2s)^T @ W3b  -> [S, 512] fp16 (for tail unpool)."""
                ps = psB3.tile([128, 4, 128], F32, tag="ps3", name="ps_ust")
                nc.tensor.matmul(
                    ps[0:S, :, :], G2s_sb[:, 0, 0:S], w3b_sb[:, 0], start=True, stop=False
                )
                nc.tensor.matmul(
                    ps[0:S, :, :], G2s_sb[:, 1, 0:S], w3b_sb[:, 1], start=False, stop=True
                )
                nc.scalar.copy(UsT_sb[0:S], ps[0:S, :, :])

            h3_tiles = {}

            def emit_L3(t):
                tail = t < T_tail
                h3_sb = h3p.tile([128, 4, PT], F16, tag="h3", name=f"h3_{t}")
                for m in range(4):
                    ps3 = psB3.tile([128, PT], F32, tag="ps3", name=f"ps3_{t}_{m}")
                    nc.tensor.matmul(
                        ps3[:, :], w3a_sb[:, 0, m, :], f2v(t)[:, 0], start=True, stop=False
                    )
                    nc.tensor.matmul(
                        ps3[:, :], w3a_sb[:, 1, m, :], f2v(t)[:, 1],
                        start=False, stop=not tail,
                    )
                    if tail:
                        nc.tensor.matmul(
                            ps3[:, :], UsT_sb[0:S, m, :],
                            onehot_sb[0:S, t * PT : (t + 1) * PT],
                            start=False, stop=True,
                        )
                        bias = b3_sb[:, m : m + 1]
                    else:
                        j = t - T_tail
                        bias = Ub_sb[:, m, j : j + 1]
                    nc.scalar.activation(out=h3_sb[:, m], in_=ps3, func=AF.Relu, bias=bias)
                h3_tiles[t] = h3_sb

            def emit_L4(t):
                tail = t < T_tail
                h3_sb = h3_tiles.pop(t)
                for mg in range(4):
                    shp = [128, 2, NG, GRP] if tail else [128, 2, PT]
                    ps4 = psB4.tile(shp, F32, tag="ps4", name=f"ps4_{t}_{mg}")
                    for mi2 in range(2):
                        for k in range(4):
                            nc.tensor.matmul(
                                ps4[:, mi2], w4_sb[:, k, 2 * mg + mi2, :], h3_sb[:, k],
                                start=(k == 0), stop=(k == 3),
                            )
                    if tail:
                        nc.vector.tensor_reduce(
                            out=Mx4_sb[:, 2 * mg : 2 * mg + 2, t * NG : (t + 1) * NG],
                            in_=ps4, axis=AXX, op=ALU_MAX,
                        )
                    else:
                        j = t - T_tail
                        nc.vector.tensor_reduce(
                            out=Mx4_sb[:, 2 * mg : 2 * mg + 2, NTC + j : NTC + j + 1],
                            in_=ps4, axis=AXX, op=ALU_MAX,
                        )
                if tail:
                    nc.sync.dma_start(
                        out=mx4.ap()[:, :, t * NG : (t + 1) * NG],
                        in_=Mx4_sb[:, :, t * NG : (t + 1) * NG],
                    )

            # ---------------- interleaved pipeline ----------------
            # B order: pure tiles (T_tail..TM-1) then tail tiles (0..T_tail-1)
            bseq = list(range(T_tail, TM)) + list(range(T_tail))

            def need_a(bj):
                t = bseq[bj]
                if t < T_tail:
                    return TM  # tail B needs every slot's g (UsT)
                j = t - T_tail
                return max(bounds[shi_pure[j]], t + 1)

            # keep the drain-paced fill as short as possible: emit only what
            # phase B's first tile needs, then let the catch-up branch (A
            # emitted while B stalls) hide later slots' A work under B
            LEAD = need_a(0)

            a_next = 0
            b_next = 0
            l3_next = 0
            g_emitted = [False] * S
            ust_emitted = False
            dmad = 0

            def try_unlock():
                nonlocal ust_emitted
                for s in range(S):
                    if not g_emitted[s] and a_next >= bounds[s]:
                        for t in [d for d in deferred_mx2 if d < bounds[s]]:
                            emit_mx2(t)
                            deferred_mx2.remove(t)
                        emit_g(s)
                        g_emitted[s] = True
                if not ust_emitted and all(g_emitted):
                    emit_UsT()
                    ust_emitted = True

            def b_ready(bj):
                t = bseq[bj]
                if t < T_tail:
                    return ust_emitted
                return g_emitted[shi_pure[t - T_tail]] and a_next > t

            while b_next < TM:
                while a_next < min(TM, b_next + LEAD):
                    emit_A(a_next, fill=(b_next == 0))
                    if b_next == 0:
                        # dummy matmuls keep the HAM clock gate open through
                        # the drain-paced fill
                        for _ in range(3):
                            nc.tensor.matmul(
                                ps_dummy[:, 0, :], warm_src[:, 0:128], warm_src[:, :],
                                start=True, stop=True,
                            )
                    a_next += 1
                    try_unlock()
                progressed = False
                while l3_next <= min(b_next + 1, TM - 1) and b_ready(l3_next):
                    emit_L3(bseq[l3_next])
                    l3_next += 1
                    progressed = True
                if l3_next > b_next:
                    if b_next == 0:
                        # bridge the prologue stall (L4(0) waits on first h3)
                        for _ in range(16):
                            nc.tensor.matmul(
                                ps_dummy[:, 0, :], warm_src[:, 0:128], warm_src[:, :],
                                start=True, stop=True,
                            )
                    # flush one deferred Mx2 per B tile into steady-state slack
                    if deferred_mx2:
                        emit_mx2(deferred_mx2.pop(0))
                    emit_L4(bseq[b_next])
                    b_next += 1
                    progressed = True
                    # stream completed pure columns out while computing
                    if b_next in (10, 20, P_pure):
                        nc.sync.dma_start(
                            out=mx4.ap()[:, :, NTC + dmad : NTC + b_next],
                            in_=Mx4_sb[:, :, NTC + dmad : NTC + b_next],
                        )
                        dmad = b_next
                if not progressed:
                    if a_next < TM:
                        emit_A(a_next, fill=(b_next == 0))
                        a_next += 1
                        try_unlock()
                    else:
                        raise RuntimeError("pipeline deadlock")

            if dmad < P_pure:
                nc.sync.dma_start(
                    out=mx4.ap()[:, :, NTC + dmad : GW], in_=Mx4_sb[:, :, NTC + dmad : GW]
                )

    nc.finalize()
    return nc


def _partition(npts: np.ndarray, n_cores: int, slots: int):
    """Assign whole segments to cores, balancing total points."""
    B = len(npts)
    order = np.argsort(-npts, kind="stable")
    best = None
    for trial in range(64):
        rng = np.random.default_rng(trial)
        seq = order.copy() if trial == 0 else rng.permutation(B)
        seq = sorted(seq, key=lambda s: -npts[s])
        if trial > 0:  # tie-break shuffles
            k = trial % 4 + 1
            seq = list(seq)
            for i in range(0, len(seq) - k, k):
                sub = seq[i : i + k]
                rng.shuffle(sub)
                seq[i : i + k] = sub
        groups = [[] for _ in range(n_cores)]
        loads = [0] * n_cores
        for s in seq:
            cands = [c for c in range(n_cores) if len(groups[c]) < slots]
            c = min(cands, key=lambda i: loads[i])
            groups[c].append(int(s))
            loads[c] += int(npts[s])
        for _ in range(400):
            hi = max(range(n_cores), key=lambda i: loads[i])
            done = True
            for lo in sorted(range(n_cores), key=lambda i: loads[i]):
                if lo == hi:
                    continue
                for ia, sa in enumerate(groups[hi]):
                    for ib, sb in enumerate(groups[lo]):
                        d = int(npts[sa]) - int(npts[sb])
                        if d > 0 and max(loads[hi] - d, loads[lo] + d) < loads[hi]:
                            groups[hi][ia], groups[lo][ib] = sb, sa
                            loads[hi] -= d
                            loads[lo] += d
                            done = False
                            break
                    if not done:
                        break
                if not done:
                    break
            if done:
                break
        key = (max(loads), tuple(sorted(loads)))
        if best is None or key < best[0]:
            best = (key, [list(g) for g in groups])
    return best[1]


def _prepare(x: np.ndarray, seg_ids: np.ndarray, B: int):
    counts = np.bincount(seg_ids, minlength=B)
    starts = np.concatenate([[0], np.cumsum(counts)])
    S = (B + N_CORES - 1) // N_CORES

    groups = _partition(counts.astype(np.int64), N_CORES, S)
    # slot 0 smallest so phase B unlocks early
    for c in range(N_CORES):
        groups[c] = sorted(groups[c], key=lambda s: counts[s])

    fulls = [sum(int(counts[s]) // PT for s in g) for g in groups]
    rem64 = [
        sum(-(-(int(counts[s]) % PT) // GRP) * GRP for s in g if int(counts[s]) % PT)
        for g in groups
    ]

    best = None
    for P_pure in range(max(1, min(fulls) - 2), max(fulls) + 1):
        T_tail = 1
        for c in range(N_CORES):
            tail_pts = rem64[c] + max(0, fulls[c] - P_pure) * PT
            T_tail = max(T_tail, -(-tail_pts // PT))
        TM = P_pure + T_tail
        if best is None or (TM, T_tail) < best[:2]:
            best = (TM, T_tail, P_pure)
    TM, T_tail, P_pure = best
    NTC = T_tail * NG
    GW = NTC + P_pure

    xT_cores, gmask_cores, umask2_cores, onehot_cores, post = [], [], [], [], []
    pure_slots_all = []
    for c in range(N_CORES):
        segs = groups[c]
        demote = max(0, fulls[c] - P_pure)
        pure_blocks = []  # (slot, pts[512,3])
        tail_parts = []  # (slot, pts[n*64,3])
        for k, s in enumerate(segs):
            pts = x[starts[s] : starts[s + 1]]
            nf = len(pts) // PT
            rem = pts[nf * PT :]
            if len(rem):
                while len(rem) % GRP:
                    rem = np.concatenate([rem, rem[: GRP - len(rem) % GRP]])
                tail_parts.append((k, rem))
            for d in range(nf):
                pure_blocks.append((k, pts[d * PT : (d + 1) * PT]))
        # demote full blocks (from the end = largest slots) into the tail
        for _ in range(demote):
            k, blk = pure_blocks.pop()
            tail_parts.append((k, blk))
        while len(pure_blocks) < P_pure:
            pure_blocks.append(pure_blocks[-1])
        assert len(pure_blocks) == P_pure, (c, len(pure_blocks), P_pure)
        if not tail_parts:
            k = len(segs) - 1
            tail_parts.append(
                (k, np.tile(x[starts[segs[k]] : starts[segs[k]] + 1], (GRP, 1)))
            )
        tail_pts = np.concatenate([p for _, p in tail_parts])
        tail_grp_slot = sum(([k] * (len(p) // GRP) for k, p in tail_parts), [])
        need = T_tail * PT - len(tail_pts)
        assert need >= 0, (c, len(tail_pts))
        if need:
            lastk = tail_parts[-1][0]
            reps = np.tile(tail_parts[-1][1][:GRP], (need // GRP, 1))
            tail_pts = np.concatenate([tail_pts, reps])
            tail_grp_slot += [lastk] * (need // GRP)
        assert len(tail_grp_slot) == NTC

        pure_slots = [k for k, _ in pure_blocks]
        pure_slots_all.append(pure_slots)

        xc = np.concatenate([tail_pts] + [p for _, p in pure_blocks]).astype(np.float16)
        xT_cores.append(np.ascontiguousarray(xc.T))

        gm = np.zeros((S, GW), np.float32)
        for col, k in enumerate(tail_grp_slot):
            gm[k, col] = 1.0
        for j, k in enumerate(pure_slots):
            gm[k, NTC + j] = 1.0
        gmask_cores.append(np.ascontiguousarray(np.broadcast_to(gm[None], (128, S, GW))))

        um = np.zeros((S, P_pure), np.float32)
        for j, k in enumerate(pure_slots):
            um[k, j] = 1.0
        umask2_cores.append(
            np.ascontiguousarray(np.broadcast_to(um[None], (128, S, P_pure)))
        )

        oh = np.zeros((S, T_tail * PT), np.float16)
        for gcol, k in enumerate(tail_grp_slot):
            oh[k, gcol * GRP : (gcol + 1) * GRP] = 1.0
        onehot_cores.append(oh)

        post.append((segs, pure_slots, tail_grp_slot))

    bounds = tuple(
        T_tail + max(
            (max((j + 1 for j, k in enumerate(psl) if k <= s), default=0))
            for psl in pure_slots_all
        )
        for s in range(S)
    )
    shi_pure = tuple(
        max(pure_slots_all[c][j] for c in range(N_CORES)) for j in range(P_pure)
    )
    return (
        (T_tail, P_pure, S, bounds, shi_pure),
        xT_cores, gmask_cores, umask2_cores, onehot_cores, post,
    )


def make_in_maps(inputs):
    x = np.asarray(inputs["x"], np.float32)
    seg_ids = np.asarray(inputs["seg_ids"])
    B = int(inputs["num_segments"])

    Wf, bf = [], []
    for i in (1, 2, 3, 4):
        W = np.asarray(inputs[f"W{i}"], np.float32)
        b = np.asarray(inputs[f"b{i}"], np.float32)
        ga = np.asarray(inputs[f"g{i}"], np.float32)
        be = np.asarray(inputs[f"be{i}"], np.float32)
        m = np.asarray(inputs[f"m{i}"], np.float32)
        v = np.asarray(inputs[f"v{i}"], np.float32)
        sc = ga / np.sqrt(v + EPS)
        Wf.append(np.ascontiguousarray(W * sc[None, :]))
        bf.append((b - m) * sc + be)
    W1f, W2f, W3f, W4f = Wf
    b1f, b2f, b3f, b4f = bf

    key, xT_cores, gmask_cores, umask2_cores, onehot_cores, post = _prepare(x, seg_ids, B)

    w1d = W1f.astype(np.float16)
    w2d = np.ascontiguousarray(W2f.reshape(128, 2, 128).astype(np.float16))
    w3ad = np.ascontiguousarray(
        W3f[:256].reshape(2, 128, 4, 128).transpose(1, 0, 2, 3).astype(np.float16)
    )
    w3bd = np.ascontiguousarray(
        W3f[256:].reshape(2, 128, 4, 128).transpose(1, 0, 2, 3).astype(np.float16)
    )
    w4d = np.ascontiguousarray(
        W4f.reshape(4, 128, 8, 128).transpose(1, 0, 2, 3).astype(np.float16)
    )
    b1d = np.ascontiguousarray(b1f.reshape(128, 1))
    b2d = np.ascontiguousarray(b2f.reshape(2, 128).T)
    b3d = np.ascontiguousarray(b3f.reshape(4, 128).T)

    in_maps = [
        {
            "xT": xT_cores[c],
            "gmask": gmask_cores[c],
            "umask2": umask2_cores[c],
            "onehot": onehot_cores[c],
            "w1": w1d,
            "w2": w2d,
            "w3a": w3ad,
            "w3b": w3bd,
            "w4": w4d,
            "b1": b1d,
            "b2": b2d,
            "b3": b3d,
        }
        for c in range(N_CORES)
    ]
    return key, in_maps, post, b4f


def postprocess(results, post, b4f, B, T_tail, P_pure):
    NTC = T_tail * NG
    out = np.zeros((B, 1024), np.float32)
    for c in range(N_CORES):
        mx4 = results[c]["mx4"]  # [128, 8, GW]
        segs, pure_slots, tail_grp_slot = post[c]
        for k, s in enumerate(segs):
            cols = [g for g, kk in enumerate(tail_grp_slot) if kk == k]
            cols += [NTC + j for j, kk in enumerate(pure_slots) if kk == k]
            raw = mx4[:, :, cols].max(axis=2)  # [128, 8]
            out[s] = np.maximum(raw.T.reshape(1024) + b4f, 0.0)
    return out


def get_program(key):
    if key not in _PROGRAM_CACHE:
        _PROGRAM_CACHE[key] = _build_program(*key)
    return _PROGRAM_CACHE[key]


def kernel(**inputs) -> np.ndarray:
    B = int(inputs["num_segments"])
    key, in_maps, post, b4f = make_in_maps(inputs)
    nc = get_program(key)
    last_err = None
    for _ in range(3):  # retry transient NRT device wedges
        try:
            res = run_bass_kernel_spmd(nc, in_maps, core_ids=list(range(N_CORES)))
            return postprocess(res.results, post, b4f, B, key[0], key[1])
        except Exception as e:  # noqa: BLE001
            last_err = e
    raise last_err


# revision 17
# speedup vs baseline: 1.0282x; 1.0282x over previous
"""PointNet-style encoder (conv1x1 stack + ragged segment-max) on 8 Trainium2 cores.

Strategy (v2.1 — tail-tile packing)
-----------------------------------
* BN folded into conv weights host-side; every layer is matmul+bias+ReLU.
* Feature-major on device: activations live as [C, points] tiles; points stream
  through the PE as the matmul free dimension in 512-point macro-tiles.
* Segments are point-balanced across the 8 cores (whole segments per core, so
  the two segment-maxes stay core-local). Per core the layout is:
    - T_TAIL fixed "tail" tiles (array idx 0..T_TAIL-1) holding every
      segment's sub-512 remainder, packed as 64-col-aligned per-slot chunks;
    - P_PURE "pure" tiles (idx T_TAIL..) each belonging to one segment.
  This wastes ~1 tile/core instead of the ~3 of per-segment padding.
* Phase A (L1+L2) runs tail tiles first, then pures in slot order; per-tile
  f2 maxes (Mx2) land in a combined column buffer: 8 per-64-col group maxes
  per tail tile, 1 per pure tile. Per-slot g = masked max over that buffer
  (gmask input zeros other slots; buffer memset 0 + f2>=0 keeps unwritten
  columns neutral).
* Mid-network unpool via concat identity: concat(f2, g)@W3 = f2@W3a + g@W3b.
  Per slot Us = W3b^T g + b3 (tiny 1-col matmuls). Pure tiles read a per-tile
  bias column Ub[:,m,j], built by one masked broadcast pass per slot (umask2
  input). Tail tiles get the g-term exactly via an extra accumulated matmul
  per m-chunk: lhsT = UsT (computed on-PE as G2^T@W3b, [S,512]), rhs =
  one-hot slot matrix O [S, cols] — per-point unpool without masks.
* Phase B (L3+L4) runs pures first (unlocked per slot as g arrives), tail
  tiles last (they need every slot's UsT). L4 maxes reduce per pure tile and
  per 64-col group for tail tiles (raw, pre-bias); the host applies
  relu(.+b4) and combines columns per segment (exact under max).
* Single interleaved pipeline (phase A runs LEAD tiles ahead of phase B) so
  A's ACT/DVE drains hide under B's PE-bound matmuls; dummy matmuls keep the
  PE HAM clock-gate open through the drain-paced fill phase.
* Matmuls in float16 (fp32 PSUM): 1 cycle/column, ~1e-3 rel err.
"""

import numpy as np

import concourse.bass as bass
import concourse.mybir as mybir
import concourse.tile as tile
from concourse import bacc
from concourse.bass_utils import run_bass_kernel_spmd

N_CORES = 8
PT = 512
GRP = 64  # tail group granularity (cols)
NG = PT // GRP  # groups per tile
EPS = 1e-3  # keras BatchNormalization default epsilon

F32 = mybir.dt.float32
F16 = mybir.dt.float16
AF = mybir.ActivationFunctionType
AXX = mybir.AxisListType.X
AXXY = mybir.AxisListType.XY
ALU_MAX = mybir.AluOpType.max
ALU_ADD = mybir.AluOpType.add

_PROGRAM_CACHE: dict = {}


def _build_program(T_tail: int, P_pure: int, S: int, bounds: tuple, shi_pure: tuple):
    """One SPMD program for all cores.

    bounds[s]: #A-tiles after which slot s's f2 is complete on every core.
    shi_pure[j]: max (over cores) slot id of pure tile j.
    """
    TM = T_tail + P_pure
    NTC = T_tail * NG  # tail group columns
    GW = NTC + P_pure  # combined max-buffer width

    nc = bacc.Bacc("TRN2")

    xT = nc.dram_tensor("xT", [3, TM * PT], F16, kind="ExternalInput")
    gmask = nc.dram_tensor("gmask", [128, S, GW], F32, kind="ExternalInput")
    umask2 = nc.dram_tensor("umask2", [128, S, P_pure], F32, kind="ExternalInput")
    onehot = nc.dram_tensor("onehot", [S, T_tail * PT], F16, kind="ExternalInput")
    w1 = nc.dram_tensor("w1", [3, 128], F16, kind="ExternalInput")
    w2 = nc.dram_tensor("w2", [128, 2, 128], F16, kind="ExternalInput")
    w3a = nc.dram_tensor("w3a", [128, 2, 4, 128], F16, kind="ExternalInput")
    w3b = nc.dram_tensor("w3b", [128, 2, 4, 128], F16, kind="ExternalInput")
    w4 = nc.dram_tensor("w4", [128, 4, 8, 128], F16, kind="ExternalInput")
    b1 = nc.dram_tensor("b1", [128, 1], F32, kind="ExternalInput")
    b2 = nc.dram_tensor("b2", [128, 2], F32, kind="ExternalInput")
    b3 = nc.dram_tensor("b3", [128, 4], F32, kind="ExternalInput")
    mx4 = nc.dram_tensor("mx4", [128, 8, GW], F32, kind="ExternalOutput")

    X_HEAD = min(6, TM)  # tiles in the first (prioritized) x DMA

    with tile.TileContext(nc) as tc:
        with (
            tc.tile_pool(name="const", bufs=1) as constp,
            tc.tile_pool(name="h1p", bufs=3) as h1p,
            tc.tile_pool(name="h3p", bufs=3) as h3p,
            tc.tile_pool(name="tmpp", bufs=4) as tmpp,
            tc.tile_pool(name="psA", bufs=2, space="PSUM") as psA,
            tc.tile_pool(name="psB3", bufs=2, space="PSUM") as psB3,
            tc.tile_pool(name="psB4", bufs=2, space="PSUM") as psB4,
        ):
            # x head first on the sync queue so the first L1 unblocks ASAP
            x_sb = constp.tile([3, TM * PT], F16)
            nc.sync.dma_start(out=x_sb[:, : X_HEAD * PT], in_=xT.ap()[:, : X_HEAD * PT])
            w1_sb = constp.tile([3, 128], F16)
            nc.sync.dma_start(out=w1_sb, in_=w1.ap())
            w2_sb = constp.tile([128, 2, 128], F16)
            nc.sync.dma_start(out=w2_sb, in_=w2.ap())
            b1_sb = constp.tile([128, 1], F32)
            nc.sync.dma_start(out=b1_sb, in_=b1.ap())
            b2_sb = constp.tile([128, 2], F32)
            nc.sync.dma_start(out=b2_sb, in_=b2.ap())
            b3_sb = constp.tile([128, 4], F32)
            nc.sync.dma_start(out=b3_sb, in_=b3.ap())
            nc.sync.dma_start(out=x_sb[:, X_HEAD * PT :], in_=xT.ap()[:, X_HEAD * PT :])
            # phase-B weights + masks on the gpsimd queue, in need order
            w3a_sb = constp.tile([128, 2, 4, 128], F16)
            nc.gpsimd.dma_start(out=w3a_sb, in_=w3a.ap())
            gmask_sb = constp.tile([128, S, GW], F32)
            nc.gpsimd.dma_start(out=gmask_sb, in_=gmask.ap())
            umask2_sb = constp.tile([128, S, P_pure], F32)
            nc.gpsimd.dma_start(out=umask2_sb, in_=umask2.ap())
            w3b_sb = constp.tile([128, 2, 4, 128], F16)
            nc.gpsimd.dma_start(out=w3b_sb, in_=w3b.ap())
            w4_sb = constp.tile([128, 4, 8, 128], F16)
            nc.gpsimd.dma_start(out=w4_sb, in_=w4.ap())
            onehot_sb = constp.tile([S, T_tail * PT], F16)
            nc.gpsimd.dma_start(out=onehot_sb, in_=onehot.ap())

            # f2 storage: tail tiles keep a grouped view for 64-col reduces
            f2_tail = constp.tile([128, T_tail, 2, NG, GRP], F16)
            f2_pure = constp.tile([128, P_pure, 2, PT], F16)
            Mx2_sb = constp.tile([128, 2, GW], F32)
            g_sb = constp.tile([128, 2, S], F32)
            G2s_sb = constp.tile([128, 2, S], F16)
            Us_sb = constp.tile([128, 4, S], F32)
            Ub_sb = constp.tile([128, 4, P_pure], F32)
            UsT_sb = constp.tile([S, 4, 128], F16)
            Mx4_sb = constp.tile([128, 8, GW], F32)

            # zero-init buffers that masked ops may read before fully written
            nc.vector.memset(Mx2_sb, 0.0)
            nc.vector.memset(Us_sb, 0.0)
            nc.vector.memset(G2s_sb, 0.0)

            # HAM warmup: dependency-free matmuls during the DMA prologue
            warm_src = constp.tile([128, PT], F16, name="warm_src")
            nc.vector.memset(warm_src, 0.01)
            warm_out = constp.tile([128, 1], F32, name="warm_out")
            warm_act = constp.tile([128, 8], F32, name="warm_act")
            nc.scalar.activation(out=warm_act, in_=warm_src[:, 0:8], func=AF.Relu)
            ps_dummy = psB4.tile([128, 2, PT], F32, tag="ps4", name="ps_warm")
            for _ in range(4):
                nc.tensor.matmul(
                    ps_dummy[:, 0, :], warm_src[:, 0:128], warm_src[:, :], start=True, stop=True
                )
            nc.vector.tensor_reduce(out=warm_out, in_=ps_dummy[:, 0, 0:8], axis=AXX, op=ALU_MAX)

            def f2v(t):
                return f2_tail[:, t] if t < T_tail else f2_pure[:, t - T_tail]

            deferred_mx2: list = []

            def emit_mx2(t):
                if t < T_tail:
                    nc.vector.tensor_reduce(
                        out=Mx2_sb[:, :, t * NG : (t + 1) * NG], in_=f2_tail[:, t],
                        axis=AXX, op=ALU_MAX,
                    )
                else:
                    j = t - T_tail
                    nc.vector.tensor_reduce(
                        out=Mx2_sb[:, :, NTC + j : NTC + j + 1], in_=f2_pure[:, j],
                        axis=AXX, op=ALU_MAX,
                    )

            def emit_A(t, fill):
                """L1+L2 for array tile t; stores f2 (fp16) + its Mx2 column(s)."""
                tail = t < T_tail
                shp = [128, NG, GRP] if tail else [128, PT]
                ps1 = psA.tile(shp, F32, tag="psa", name=f"ps1_{t}")
                nc.tensor.matmul(
                    ps1[:, :], w1_sb[:, :], x_sb[:, t * PT : (t + 1) * PT],
                    start=True, stop=True,
                )
                h1_sb = h1p.tile(shp, F16, tag="h1", name=f"h1_{t}")
                nc.scalar.activation(out=h1_sb, in_=ps1, func=AF.Relu, bias=b1_sb[:, 0:1])
                for c in range(2):
                    ps2 = psA.tile(shp, F32, tag="psa", name=f"ps2_{t}_{c}")
                    nc.tensor.matmul(ps2[:, :], w2_sb[:, c, :], h1_sb[:, :], start=True, stop=True)
                    # engine split: fill is drain-paced, steady state hides
                    # drains under phase B; Mx2 for pre-bounds[0] tiles must
                    # run inline (slot 0 unlock), later fill tiles defer
                    if fill and t >= bounds[0]:
                        on_dve = True  # both chunks on DVE, no Mx2 inline
                    else:
                        on_dve = c == 1 and fill
                    if not fill:
                        on_dve = False  # steady: ACT takes all three drains
                    if on_dve:
                        nc.vector.tensor_scalar(
                            f2v(t)[:, c], ps2, b2_sb[:, c : c + 1], 0.0, ALU_ADD, ALU_MAX
                        )
                    else:
                        nc.scalar.activation(
                            out=f2v(t)[:, c], in_=ps2, func=AF.Relu, bias=b2_sb[:, c : c + 1]
                        )
                if fill and t >= bounds[0]:
                    deferred_mx2.append(t)
                else:
                    emit_mx2(t)

            def emit_g(s):
                """g[s] = masked max over the Mx2 buffer; then Us[s], Ub pass."""
                for c in range(2):
                    tmp = tmpp.tile([128, GW], F32, tag="tmp", name=f"tmpg_{c}_{s}")
                    nc.vector.tensor_mul(tmp[:, :], Mx2_sb[:, c, :], gmask_sb[:, s, :])
                    nc.vector.tensor_reduce(
                        out=g_sb[:, c, s : s + 1], in_=tmp[:, :], axis=AXX, op=ALU_MAX
                    )
                nc.scalar.copy(G2s_sb[:, :, s], g_sb[:, :, s])
                psu = psA.tile([128, PT], F32, tag="psa", name=f"psu_{s}")
                for m in range(4):
                    nc.tensor.matmul(
                        psu[:, m : m + 1], w3b_sb[:, 0, m, :], G2s_sb[:, 0, s : s + 1],
                        start=True, stop=False,
                    )
                    nc.tensor.matmul(
                        psu[:, m : m + 1], w3b_sb[:, 1, m, :], G2s_sb[:, 1, s : s + 1],
                        start=False, stop=True,
                    )
                for m in range(4):
                    nc.scalar.activation(
                        out=Us_sb[:, m, s : s + 1], in_=psu[:, m : m + 1],
                        func=AF.Identity, bias=b3_sb[:, m : m + 1],
                    )
                # broadcast Us[slot] into the per-tile bias table
                for m in range(4):
                    if s == 0:
                        nc.vector.tensor_scalar_mul(
                            Ub_sb[:, m, :], umask2_sb[:, s, :], Us_sb[:, m, s : s + 1]
                        )
                    else:
                        tmpu = tmpp.tile([128, P_pure], F32, tag="tmpu", name=f"tmpu_{s}_{m}")
                        nc.vector.tensor_scalar_mul(
                            tmpu[:, :], umask2_sb[:, s, :], Us_sb[:, m, s : s + 1]
                        )
                        nc.vector.tensor_add(Ub_sb[:, m, :], Ub_sb[:, m, :], tmpu[:, :])

            def emit_UsT():
                """UsT = (G2s)^T @ W3b  -> [S, 512] fp16 (for tail unpool)."""
                ps = psB3.tile([128, 4, 128], F32, tag="ps3", name="ps_ust")
                nc.tensor.matmul(
                    ps[0:S, :, :], G2s_sb[:, 0, 0:S], w3b_sb[:, 0], start=True, stop=False
                )
                nc.tensor.matmul(
                    ps[0:S, :, :], G2s_sb[:, 1, 0:S], w3b_sb[:, 1], start=False, stop=True
                )
                nc.scalar.copy(UsT_sb[0:S], ps[0:S, :, :])

            h3_tiles = {}

            def emit_L3(t):
                tail = t < T_tail
                h3_sb = h3p.tile([128, 4, PT], F16, tag="h3", name=f"h3_{t}")
                for m in range(4):
                    ps3 = psB3.tile([128, PT], F32, tag="ps3", name=f"ps3_{t}_{m}")
                    nc.tensor.matmul(
                        ps3[:, :], w3a_sb[:, 0, m, :], f2v(t)[:, 0], start=True, stop=False
                    )
                    nc.tensor.matmul(
                        ps3[:, :], w3a_sb[:, 1, m, :], f2v(t)[:, 1],
                        start=False, stop=not tail,
                    )
                    if tail:
                        nc.tensor.matmul(
                            ps3[:, :], UsT_sb[0:S, m, :],
                            onehot_sb[0:S, t * PT : (t + 1) * PT],
                            start=False, stop=True,
                        )
                        bias = b3_sb[:, m : m + 1]
                    else:
                        j = t - T_tail
                        bias = Ub_sb[:, m, j : j + 1]
                    nc.scalar.activation(out=h3_sb[:, m], in_=ps3, func=AF.Relu, bias=bias)
                h3_tiles[t] = h3_sb

            def emit_L4(t):
                tail = t < T_tail
                h3_sb = h3_tiles.pop(t)
                for mg in range(4):
                    shp = [128, 2, NG, GRP] if tail else [128, 2, PT]
                    ps4 = psB4.tile(shp, F32, tag="ps4", name=f"ps4_{t}_{mg}")
                    for mi2 in range(2):
                        for k in range(4):
                            nc.tensor.matmul(
                                ps4[:, mi2], w4_sb[:, k, 2 * mg + mi2, :], h3_sb[:, k],
                                start=(k == 0), stop=(k == 3),
                            )
                    if tail:
                        nc.vector.tensor_reduce(
                            out=Mx4_sb[:, 2 * mg : 2 * mg + 2, t * NG : (t + 1) * NG],
                            in_=ps4, axis=AXX, op=ALU_MAX,
                        )
                    else:
                        j = t - T_tail
                        nc.vector.tensor_reduce(
                            out=Mx4_sb[:, 2 * mg : 2 * mg + 2, NTC + j : NTC + j + 1],
                            in_=ps4, axis=AXX, op=ALU_MAX,
                        )

            # ---------------- interleaved pipeline ----------------
            # B order: pure tiles (T_tail..TM-1) then tail tiles (0..T_tail-1)
            bseq = list(range(T_tail, TM)) + list(range(T_tail))

            def need_a(bj):
                t = bseq[bj]
                if t < T_tail:
                    return TM  # tail B needs every slot's g (UsT)
                j = t - T_tail
                return max(bounds[shi_pure[j]], t + 1)

            # keep the drain-paced fill as short as possible: emit only what
            # phase B's first tile needs, then let the catch-up branch (A
            # emitted while B stalls) hide later slots' A work under B
            LEAD = need_a(0)

            a_next = 0
            b_next = 0
            l3_next = 0
            g_emitted = [False] * S
            ust_emitted = False

            def try_unlock():
                nonlocal ust_emitted
                for s in range(S):
                    if not g_emitted[s] and a_next >= bounds[s]:
                        for t in [d for d in deferred_mx2 if d < bounds[s]]:
                            emit_mx2(t)
                            deferred_mx2.remove(t)
                        emit_g(s)
                        g_emitted[s] = True
                if not ust_emitted and all(g_emitted):
                    emit_UsT()
                    ust_emitted = True

            def b_ready(bj):
                t = bseq[bj]
                if t < T_tail:
                    return ust_emitted
                return g_emitted[shi_pure[t - T_tail]] and a_next > t

            while b_next < TM:
                # A-emission target: steady lead, plus pre-emission of what
                # B needs a few tiles out so slot unlocks never burst-emit
                # several drain-heavy A tiles back to back
                target = min(TM, max(b_next + LEAD, need_a(min(b_next + 3, TM - 1))))
                cap = TM if b_next == 0 else 2
                while a_next < target and cap > 0:
                    emit_A(a_next, fill=(b_next == 0))
                    if b_next == 0:
                        # dummy matmuls keep the HAM clock gate open through
                        # the drain-paced fill
                        for _ in range(3):
                            nc.tensor.matmul(
                                ps_dummy[:, 0, :], warm_src[:, 0:128], warm_src[:, :],
                                start=True, stop=True,
                            )
                    a_next += 1
                    cap -= 1
                    try_unlock()
                progressed = False
                while l3_next <= min(b_next + 1, TM - 1) and b_ready(l3_next):
                    emit_L3(bseq[l3_next])
                    l3_next += 1
                    progressed = True
                if l3_next > b_next:
                    if b_next == 0:
                        # bridge the prologue stall (L4(0) waits on first h3)
                        for _ in range(16):
                            nc.tensor.matmul(
                                ps_dummy[:, 0, :], warm_src[:, 0:128], warm_src[:, :],
                                start=True, stop=True,
                            )
                    # flush one deferred Mx2 per B tile into steady-state slack
                    if deferred_mx2:
                        emit_mx2(deferred_mx2.pop(0))
                    emit_L4(bseq[b_next])
                    b_next += 1
                    progressed = True
                if not progressed:
                    if a_next < TM:
                        emit_A(a_next, fill=(b_next == 0))
                        a_next += 1
                        try_unlock()
                    else:
                        raise RuntimeError("pipeline deadlock")

            nc.sync.dma_start(out=mx4.ap(), in_=Mx4_sb)

    nc.finalize()
    return nc


def _partition(npts: np.ndarray, n_cores: int, slots: int):
    """Assign whole segments to cores, balancing total points."""
    B = len(npts)
    order = np.argsort(-npts, kind="stable")
    best = None
    for trial in range(64):
        rng = np.random.default_rng(trial)
        seq = order.copy() if trial == 0 else rng.permutation(B)
        seq = sorted(seq, key=lambda s: -npts[s])
        if trial > 0:  # tie-break shuffles
            k = trial % 4 + 1
            seq = list(seq)
            for i in range(0, len(seq) - k, k):
                sub = seq[i : i + k]
                rng.shuffle(sub)
                seq[i : i + k] = sub
        groups = [[] for _ in range(n_cores)]
        loads = [0] * n_cores
        for s in seq:
            cands = [c for c in range(n_cores) if len(groups[c]) < slots]
            c = min(cands, key=lambda i: loads[i])
            groups[c].append(int(s))
            loads[c] += int(npts[s])
        for _ in range(400):
            hi = max(range(n_cores), key=lambda i: loads[i])
            done = True
            for lo in sorted(range(n_cores), key=lambda i: loads[i]):
                if lo == hi:
                    continue
                for ia, sa in enumerate(groups[hi]):
                    for ib, sb in enumerate(groups[lo]):
                        d = int(npts[sa]) - int(npts[sb])
                        if d > 0 and max(loads[hi] - d, loads[lo] + d) < loads[hi]:
                            groups[hi][ia], groups[lo][ib] = sb, sa
                            loads[hi] -= d
                            loads[lo] += d
                            done = False
                            break
                    if not done:
                        break
                if not done:
                    break
            if done:
                break
        key = (max(loads), tuple(sorted(loads)))
        if best is None or key < best[0]:
            best = (key, [list(g) for g in groups])
    return best[1]


def _prepare(x: np.ndarray, seg_ids: np.ndarray, B: int):
    counts = np.bincount(seg_ids, minlength=B)
    starts = np.concatenate([[0], np.cumsum(counts)])
    S = (B + N_CORES - 1) // N_CORES

    groups = _partition(counts.astype(np.int64), N_CORES, S)
    # slot 0 smallest so phase B unlocks early
    for c in range(N_CORES):
        groups[c] = sorted(groups[c], key=lambda s: counts[s])

    fulls = [sum(int(counts[s]) // PT for s in g) for g in groups]
    rem64 = [
        sum(-(-(int(counts[s]) % PT) // GRP) * GRP for s in g if int(counts[s]) % PT)
        for g in groups
    ]

    best = None
    for P_pure in range(max(1, min(fulls) - 2), max(fulls) + 1):
        T_tail = 1
        for c in range(N_CORES):
            tail_pts = rem64[c] + max(0, fulls[c] - P_pure) * PT
            T_tail = max(T_tail, -(-tail_pts // PT))
        TM = P_pure + T_tail
        if best is None or (TM, T_tail) < best[:2]:
            best = (TM, T_tail, P_pure)
    TM, T_tail, P_pure = best
    NTC = T_tail * NG
    GW = NTC + P_pure

    xT_cores, gmask_cores, umask2_cores, onehot_cores, post = [], [], [], [], []
    pure_slots_all = []
    for c in range(N_CORES):
        segs = groups[c]
        demote = max(0, fulls[c] - P_pure)
        pure_blocks = []  # (slot, pts[512,3])
        tail_parts = []  # (slot, pts[n*64,3])
        for k, s in enumerate(segs):
            pts = x[starts[s] : starts[s + 1]]
            nf = len(pts) // PT
            rem = pts[nf * PT :]
            if len(rem):
                while len(rem) % GRP:
                    rem = np.concatenate([rem, rem[: GRP - len(rem) % GRP]])
                tail_parts.append((k, rem))
            for d in range(nf):
                pure_blocks.append((k, pts[d * PT : (d + 1) * PT]))
        # demote full blocks (from the end = largest slots) into the tail
        for _ in range(demote):
            k, blk = pure_blocks.pop()
            tail_parts.append((k, blk))
        while len(pure_blocks) < P_pure:
            pure_blocks.append(pure_blocks[-1])
        assert len(pure_blocks) == P_pure, (c, len(pure_blocks), P_pure)
        if not tail_parts:
            k = len(segs) - 1
            tail_parts.append(
                (k, np.tile(x[starts[segs[k]] : starts[segs[k]] + 1], (GRP, 1)))
            )
        tail_pts = np.concatenate([p for _, p in tail_parts])
        tail_grp_slot = sum(([k] * (len(p) // GRP) for k, p in tail_parts), [])
        need = T_tail * PT - len(tail_pts)
        assert need >= 0, (c, len(tail_pts))
        if need:
            lastk = tail_parts[-1][0]
            reps = np.tile(tail_parts[-1][1][:GRP], (need // GRP, 1))
            tail_pts = np.concatenate([tail_pts, reps])
            tail_grp_slot += [lastk] * (need // GRP)
        assert len(tail_grp_slot) == NTC

        pure_slots = [k for k, _ in pure_blocks]
        pure_slots_all.append(pure_slots)

        xc = np.concatenate([tail_pts] + [p for _, p in pure_blocks]).astype(np.float16)
        xT_cores.append(np.ascontiguousarray(xc.T))

        gm = np.zeros((S, GW), np.float32)
        for col, k in enumerate(tail_grp_slot):
            gm[k, col] = 1.0
        for j, k in enumerate(pure_slots):
            gm[k, NTC + j] = 1.0
        gmask_cores.append(np.ascontiguousarray(np.broadcast_to(gm[None], (128, S, GW))))

        um = np.zeros((S, P_pure), np.float32)
        for j, k in enumerate(pure_slots):
            um[k, j] = 1.0
        umask2_cores.append(
            np.ascontiguousarray(np.broadcast_to(um[None], (128, S, P_pure)))
        )

        oh = np.zeros((S, T_tail * PT), np.float16)
        for gcol, k in enumerate(tail_grp_slot):
            oh[k, gcol * GRP : (gcol + 1) * GRP] = 1.0
        onehot_cores.append(oh)

        post.append((segs, pure_slots, tail_grp_slot))

    bounds = tuple(
        T_tail + max(
            (max((j + 1 for j, k in enumerate(psl) if k <= s), default=0))
            for psl in pure_slots_all
        )
        for s in range(S)
    )
    shi_pure = tuple(
        max(pure_slots_all[c][j] for c in range(N_CORES)) for j in range(P_pure)
    )
    return (
        (T_tail, P_pure, S, bounds, shi_pure),
        xT_cores, gmask_cores, umask2_cores, onehot_cores, post,
    )


def make_in_maps(inputs):
    x = np.asarray(inputs["x"], np.float32)
    seg_ids = np.asarray(inputs["seg_ids"])
    B = int(inputs["num_segments"])

    Wf, bf = [], []
    for i in (1, 2, 3, 4):
        W = np.asarray(inputs[f"W{i}"], np.float32)
        b = np.asarray(inputs[f"b{i}"], np.float32)
        ga = np.asarray(inputs[f"g{i}"], np.float32)
        be = np.asarray(inputs[f"be{i}"], np.float32)
        m = np.asarray(inputs[f"m{i}"], np.float32)
        v = np.asarray(inputs[f"v{i}"], np.float32)
        sc = ga / np.sqrt(v + EPS)
        Wf.append(np.ascontiguousarray(W * sc[None, :]))
        bf.append((b - m) * sc + be)
    W1f, W2f, W3f, W4f = Wf
    b1f, b2f, b3f, b4f = bf

    key, xT_cores, gmask_cores, umask2_cores, onehot_cores, post = _prepare(x, seg_ids, B)

    w1d = W1f.astype(np.float16)
    w2d = np.ascontiguousarray(W2f.reshape(128, 2, 128).astype(np.float16))
    w3ad = np.ascontiguousarray(
        W3f[:256].reshape(2, 128, 4, 128).transpose(1, 0, 2, 3).astype(np.float16)
    )
    w3bd = np.ascontiguousarray(
        W3f[256:].reshape(2, 128, 4, 128).transpose(1, 0, 2, 3).astype(np.float16)
    )
    w4d = np.ascontiguousarray(
        W4f.reshape(4, 128, 8, 128).transpose(1, 0, 2, 3).astype(np.float16)
    )
    b1d = np.ascontiguousarray(b1f.reshape(128, 1))
    b2d = np.ascontiguousarray(b2f.reshape(2, 128).T)
    b3d = np.ascontiguousarray(b3f.reshape(4, 128).T)

    in_maps = [
        {
            "xT": xT_cores[c],
            "gmask": gmask_cores[c],
            "umask2": umask2_cores[c],
            "onehot": onehot_cores[c],
            "w1": w1d,
            "w2": w2d,
            "w3a": w3ad,
            "w3b": w3bd,
            "w4": w4d,
            "b1": b1d,
            "b2": b2d,
            "b3": b3d,
        }
        for c in range(N_CORES)
    ]
    return key, in_maps, post, b4f


def postprocess(results, post, b4f, B, T_tail, P_pure):
    NTC = T_tail * NG
    out = np.zeros((B, 1024), np.float32)
    for c in range(N_CORES):
        mx4 = results[c]["mx4"]  # [128, 8, GW]
        segs, pure_slots, tail_grp_slot = post[c]
        for k, s in enumerate(segs):
            cols = [g for g, kk in enumerate(tail_grp_slot) if kk == k]
            cols += [NTC + j for j, kk in enumerate(pure_slots) if kk == k]
            raw = mx4[:, :, cols].max(axis=2)  # [128, 8]
            out[s] = np.maximum(raw.T.reshape(1024) + b4f, 0.0)
    return out


def get_program(key):
    if key not in _PROGRAM_CACHE:
        _PROGRAM_CACHE[key] = _build_program(*key)
    return _PROGRAM_CACHE[key]


def kernel(**inputs) -> np.ndarray:
    B = int(inputs["num_segments"])
    key, in_maps, post, b4f = make_in_maps(inputs)
    nc = get_program(key)
    last_err = None
    for _ in range(3):  # retry transient NRT device wedges
        try:
            res = run_bass_kernel_spmd(nc, in_maps, core_ids=list(range(N_CORES)))
            return postprocess(res.results, post, b4f, B, key[0], key[1])
        except Exception as e:  # noqa: BLE001
            last_err = e
    raise last_err


# revision 18
# speedup vs baseline: 1.0850x; 1.0552x over previous
"""PointNet-style encoder (conv1x1 stack + ragged segment-max) on 8 Trainium2 cores.

Strategy (v2.1 — tail-tile packing)
-----------------------------------
* BN folded into conv weights host-side; every layer is matmul+bias+ReLU.
* Feature-major on device: activations live as [C, points] tiles; points stream
  through the PE as the matmul free dimension in 512-point macro-tiles.
* Segments are point-balanced across the 8 cores (whole segments per core, so
  the two segment-maxes stay core-local). Per core the layout is:
    - T_TAIL fixed "tail" tiles (array idx 0..T_TAIL-1) holding every
      segment's sub-512 remainder, packed as 64-col-aligned per-slot chunks;
    - P_PURE "pure" tiles (idx T_TAIL..) each belonging to one segment.
  This wastes ~1 tile/core instead of the ~3 of per-segment padding.
* Phase A (L1+L2) runs tail tiles first, then pures in slot order; per-tile
  f2 maxes (Mx2) land in a combined column buffer: 8 per-64-col group maxes
  per tail tile, 1 per pure tile. Per-slot g = masked max over that buffer
  (gmask input zeros other slots; buffer memset 0 + f2>=0 keeps unwritten
  columns neutral).
* Mid-network unpool via concat identity: concat(f2, g)@W3 = f2@W3a + g@W3b.
  Per slot Us = W3b^T g + b3 (tiny 1-col matmuls). Pure tiles read a per-tile
  bias column Ub[:,m,j], built by one masked broadcast pass per slot (umask2
  input). Tail tiles get the g-term exactly via an extra accumulated matmul
  per m-chunk: lhsT = UsT (computed on-PE as G2^T@W3b, [S,512]), rhs =
  one-hot slot matrix O [S, cols] — per-point unpool without masks.
* Phase B (L3+L4) runs pures first (unlocked per slot as g arrives), tail
  tiles last (they need every slot's UsT). L4 maxes reduce per pure tile and
  per 64-col group for tail tiles (raw, pre-bias); the host applies
  relu(.+b4) and combines columns per segment (exact under max).
* Single interleaved pipeline (phase A runs LEAD tiles ahead of phase B) so
  A's ACT/DVE drains hide under B's PE-bound matmuls; dummy matmuls keep the
  PE HAM clock-gate open through the drain-paced fill phase.
* Matmuls in float16 (fp32 PSUM): 1 cycle/column, ~1e-3 rel err.
"""

import numpy as np

import concourse.bass as bass
import concourse.mybir as mybir
import concourse.tile as tile
from concourse import bacc
from concourse.bass_utils import run_bass_kernel_spmd

N_CORES = 8
PT = 512
GRP = 64  # tail group granularity (cols)
NG = PT // GRP  # groups per tile
EPS = 1e-3  # keras BatchNormalization default epsilon

F32 = mybir.dt.float32
F16 = mybir.dt.float16
AF = mybir.ActivationFunctionType
AXX = mybir.AxisListType.X
AXXY = mybir.AxisListType.XY
ALU_MAX = mybir.AluOpType.max
ALU_ADD = mybir.AluOpType.add

_PROGRAM_CACHE: dict = {}


def _build_program(T_tail: int, P_pure: int, S: int, bounds: tuple, shi_pure: tuple):
    """One SPMD program for all cores.

    bounds[s]: #A-tiles after which slot s's f2 is complete on every core.
    shi_pure[j]: max (over cores) slot id of pure tile j.
    """
    TM = T_tail + P_pure
    NTC = T_tail * NG  # tail group columns
    GW = NTC + P_pure  # combined max-buffer width

    nc = bacc.Bacc("TRN2")

    xT = nc.dram_tensor("xT", [3, TM * PT], F16, kind="ExternalInput")
    gmask = nc.dram_tensor("gmask", [128, S, GW], F32, kind="ExternalInput")
    umask2 = nc.dram_tensor("umask2", [128, S, P_pure], F32, kind="ExternalInput")
    onehot = nc.dram_tensor("onehot", [S, T_tail * PT], F16, kind="ExternalInput")
    w1 = nc.dram_tensor("w1", [3, 128], F16, kind="ExternalInput")
    w2 = nc.dram_tensor("w2", [128, 2, 128], F16, kind="ExternalInput")
    w3a = nc.dram_tensor("w3a", [128, 2, 4, 128], F16, kind="ExternalInput")
    w3b = nc.dram_tensor("w3b", [128, 2, 4, 128], F16, kind="ExternalInput")
    w4 = nc.dram_tensor("w4", [128, 4, 8, 128], F16, kind="ExternalInput")
    b1 = nc.dram_tensor("b1", [128, 1], F32, kind="ExternalInput")
    b2 = nc.dram_tensor("b2", [128, 2], F32, kind="ExternalInput")
    b3 = nc.dram_tensor("b3", [128, 4], F32, kind="ExternalInput")
    mx4 = nc.dram_tensor("mx4", [128, 8, GW], F32, kind="ExternalOutput")

    X_HEAD = min(6, TM)  # tiles in the first (prioritized) x DMA

    with tile.TileContext(nc) as tc:
        with (
            tc.tile_pool(name="const", bufs=1) as constp,
            tc.tile_pool(name="h1p", bufs=3) as h1p,
            tc.tile_pool(name="h3p", bufs=3) as h3p,
            tc.tile_pool(name="tmpp", bufs=4) as tmpp,
            tc.tile_pool(name="psA", bufs=4, space="PSUM") as psA,
            tc.tile_pool(name="psB3", bufs=2, space="PSUM") as psB3,
            tc.tile_pool(name="psB4", bufs=2, space="PSUM") as psB4,
        ):
            # x head first on the sync queue so the first L1 unblocks ASAP
            x_sb = constp.tile([3, TM * PT], F16)
            nc.sync.dma_start(out=x_sb[:, : 2 * PT], in_=xT.ap()[:, : 2 * PT])
            nc.sync.dma_start(
                out=x_sb[:, 2 * PT : X_HEAD * PT], in_=xT.ap()[:, 2 * PT : X_HEAD * PT]
            )
            w1_sb = constp.tile([3, 128], F16)
            nc.sync.dma_start(out=w1_sb, in_=w1.ap())
            w2_sb = constp.tile([128, 2, 128], F16)
            nc.sync.dma_start(out=w2_sb, in_=w2.ap())
            b1_sb = constp.tile([128, 1], F32)
            nc.sync.dma_start(out=b1_sb, in_=b1.ap())
            b2_sb = constp.tile([128, 2], F32)
            nc.sync.dma_start(out=b2_sb, in_=b2.ap())
            b3_sb = constp.tile([128, 4], F32)
            nc.sync.dma_start(out=b3_sb, in_=b3.ap())
            nc.sync.dma_start(out=x_sb[:, X_HEAD * PT :], in_=xT.ap()[:, X_HEAD * PT :])
            # phase-B weights + masks on the gpsimd queue, in need order
            w3a_sb = constp.tile([128, 2, 4, 128], F16)
            nc.gpsimd.dma_start(out=w3a_sb, in_=w3a.ap())
            gmask_sb = constp.tile([128, S, GW], F32)
            nc.gpsimd.dma_start(out=gmask_sb, in_=gmask.ap())
            umask2_sb = constp.tile([128, S, P_pure], F32)
            nc.gpsimd.dma_start(out=umask2_sb, in_=umask2.ap())
            w3b_sb = constp.tile([128, 2, 4, 128], F16)
            nc.gpsimd.dma_start(out=w3b_sb, in_=w3b.ap())
            w4_sb = constp.tile([128, 4, 8, 128], F16)
            nc.gpsimd.dma_start(out=w4_sb, in_=w4.ap())
            onehot_sb = constp.tile([S, T_tail * PT], F16)
            nc.gpsimd.dma_start(out=onehot_sb, in_=onehot.ap())

            # f2 storage: tail tiles keep a grouped view for 64-col reduces
            f2_tail = constp.tile([128, T_tail, 2, NG, GRP], F16)
            f2_pure = constp.tile([128, P_pure, 2, PT], F16)
            Mx2_sb = constp.tile([128, 2, GW], F32)
            g_sb = constp.tile([128, 2, S], F32)
            G2s_sb = constp.tile([128, 2, S], F16)
            Us_sb = constp.tile([128, 4, S], F32)
            Ub_sb = constp.tile([128, 4, P_pure], F32)
            UsT_sb = constp.tile([S, 4, 128], F16)
            Mx4_sb = constp.tile([128, 8, GW], F32)

            # zero-init buffers that masked ops may read before fully written
            nc.vector.memset(Mx2_sb, 0.0)
            nc.vector.memset(Us_sb, 0.0)
            nc.vector.memset(G2s_sb, 0.0)

            # HAM warmup: dependency-free matmuls during the DMA prologue
            warm_src = constp.tile([128, PT], F16, name="warm_src")
            nc.vector.memset(warm_src, 0.01)
            warm_out = constp.tile([128, 1], F32, name="warm_out")
            warm_act = constp.tile([128, 8], F32, name="warm_act")
            nc.scalar.activation(out=warm_act, in_=warm_src[:, 0:8], func=AF.Relu)
            ps_dummy = psB4.tile([128, PT], F32, tag="ps4", name="ps_warm")
            for _ in range(5):
                nc.tensor.matmul(
                    ps_dummy[:, :], warm_src[:, 0:128], warm_src[:, :], start=True, stop=True
                )
            nc.vector.tensor_reduce(out=warm_out, in_=ps_dummy[:, 0:8], axis=AXX, op=ALU_MAX)

            def f2v(t):
                return f2_tail[:, t] if t < T_tail else f2_pure[:, t - T_tail]

            deferred_mx2: list = []

            def emit_mx2(t):
                if t < T_tail:
                    nc.vector.tensor_reduce(
                        out=Mx2_sb[:, :, t * NG : (t + 1) * NG], in_=f2_tail[:, t],
                        axis=AXX, op=ALU_MAX,
                    )
                else:
                    j = t - T_tail
                    nc.vector.tensor_reduce(
                        out=Mx2_sb[:, :, NTC + j : NTC + j + 1], in_=f2_pure[:, j],
                        axis=AXX, op=ALU_MAX,
                    )

            def emit_A(t, fill):
                """L1+L2 for array tile t; stores f2 (fp16) + its Mx2 column(s)."""
                tail = t < T_tail
                shp = [128, NG, GRP] if tail else [128, PT]
                ps1 = psA.tile(shp, F32, tag="psa", name=f"ps1_{t}")
                nc.tensor.matmul(
                    ps1[:, :], w1_sb[:, :], x_sb[:, t * PT : (t + 1) * PT],
                    start=True, stop=True,
                )
                h1_sb = h1p.tile(shp, F16, tag="h1", name=f"h1_{t}")
                nc.scalar.activation(out=h1_sb, in_=ps1, func=AF.Relu, bias=b1_sb[:, 0:1])
                for c in range(2):
                    ps2 = psA.tile(shp, F32, tag="psa", name=f"ps2_{t}_{c}")
                    nc.tensor.matmul(ps2[:, :], w2_sb[:, c, :], h1_sb[:, :], start=True, stop=True)
                    # engine split: fill is drain-paced, steady state hides
                    # drains under phase B; Mx2 for pre-bounds[0] tiles must
                    # run inline (slot 0 unlock), later fill tiles defer
                    if fill and t >= bounds[0]:
                        on_dve = True  # both chunks on DVE, no Mx2 inline
                    else:
                        on_dve = c == 1 and fill
                    if not fill:
                        on_dve = False  # steady: ACT takes all three drains
                    if on_dve:
                        nc.vector.tensor_scalar(
                            f2v(t)[:, c], ps2, b2_sb[:, c : c + 1], 0.0, ALU_ADD, ALU_MAX
                        )
                    else:
                        nc.scalar.activation(
                            out=f2v(t)[:, c], in_=ps2, func=AF.Relu, bias=b2_sb[:, c : c + 1]
                        )
                if fill and t >= bounds[0]:
                    deferred_mx2.append(t)
                else:
                    emit_mx2(t)

            def emit_g(s):
                """g[s] = masked max over the Mx2 buffer; then Us[s], Ub pass."""
                for c in range(2):
                    tmp = tmpp.tile([128, GW], F32, tag="tmp", name=f"tmpg_{c}_{s}")
                    nc.vector.tensor_mul(tmp[:, :], Mx2_sb[:, c, :], gmask_sb[:, s, :])
                    nc.vector.tensor_reduce(
                        out=g_sb[:, c, s : s + 1], in_=tmp[:, :], axis=AXX, op=ALU_MAX
                    )
                nc.scalar.copy(G2s_sb[:, :, s], g_sb[:, :, s])
                psu = psA.tile([128, PT], F32, tag="psa", name=f"psu_{s}")
                for m in range(4):
                    nc.tensor.matmul(
                        psu[:, m : m + 1], w3b_sb[:, 0, m, :], G2s_sb[:, 0, s : s + 1],
                        start=True, stop=False,
                    )
                    nc.tensor.matmul(
                        psu[:, m : m + 1], w3b_sb[:, 1, m, :], G2s_sb[:, 1, s : s + 1],
                        start=False, stop=True,
                    )
                for m in range(4):
                    nc.scalar.activation(
                        out=Us_sb[:, m, s : s + 1], in_=psu[:, m : m + 1],
                        func=AF.Identity, bias=b3_sb[:, m : m + 1],
                    )
                # broadcast Us[slot] into the per-tile bias table
                for m in range(4):
                    if s == 0:
                        nc.vector.tensor_scalar_mul(
                            Ub_sb[:, m, :], umask2_sb[:, s, :], Us_sb[:, m, s : s + 1]
                        )
                    else:
                        tmpu = tmpp.tile([128, P_pure], F32, tag="tmpu", name=f"tmpu_{s}_{m}")
                        nc.vector.tensor_scalar_mul(
                            tmpu[:, :], umask2_sb[:, s, :], Us_sb[:, m, s : s + 1]
                        )
                        nc.vector.tensor_add(Ub_sb[:, m, :], Ub_sb[:, m, :], tmpu[:, :])

            def emit_UsT():
                """UsT = (G2s)^T @ W3b  -> [S, 512] fp16 (for tail unpool)."""
                ps = psB3.tile([128, 4, 128], F32, tag="ps3", name="ps_ust")
                nc.tensor.matmul(
                    ps[0:S, :, :], G2s_sb[:, 0, 0:S], w3b_sb[:, 0], start=True, stop=False
                )
                nc.tensor.matmul(
                    ps[0:S, :, :], G2s_sb[:, 1, 0:S], w3b_sb[:, 1], start=False, stop=True
                )
                nc.scalar.copy(UsT_sb[0:S], ps[0:S, :, :])

            h3_tiles = {}

            def emit_L3(t):
                tail = t < T_tail
                h3_sb = h3p.tile([128, 4, PT], F16, tag="h3", name=f"h3_{t}")
                for m in range(4):
                    ps3 = psB3.tile([128, PT], F32, tag="ps3", name=f"ps3_{t}_{m}")
                    nc.tensor.matmul(
                        ps3[:, :], w3a_sb[:, 0, m, :], f2v(t)[:, 0], start=True, stop=False
                    )
                    nc.tensor.matmul(
                        ps3[:, :], w3a_sb[:, 1, m, :], f2v(t)[:, 1],
                        start=False, stop=not tail,
                    )
                    if tail:
                        nc.tensor.matmul(
                            ps3[:, :], UsT_sb[0:S, m, :],
                            onehot_sb[0:S, t * PT : (t + 1) * PT],
                            start=False, stop=True,
                        )
                        bias = b3_sb[:, m : m + 1]
                    else:
                        j = t - T_tail
                        bias = Ub_sb[:, m, j : j + 1]
                    nc.scalar.activation(out=h3_sb[:, m], in_=ps3, func=AF.Relu, bias=bias)
                h3_tiles[t] = h3_sb

            def emit_L4(t):
                tail = t < T_tail
                h3_sb = h3_tiles.pop(t)
                for mi in range(8):
                    shp = [128, NG, GRP] if tail else [128, PT]
                    ps4 = psB4.tile(shp, F32, tag="ps4", name=f"ps4_{t}_{mi}")
                    for k in range(4):
                        nc.tensor.matmul(
                            ps4[:, :], w4_sb[:, k, mi, :], h3_sb[:, k],
                            start=(k == 0), stop=(k == 3),
                        )
                    if tail:
                        nc.vector.tensor_reduce(
                            out=Mx4_sb[:, mi, t * NG : (t + 1) * NG], in_=ps4,
                            axis=AXX, op=ALU_MAX,
                        )
                    else:
                        j = t - T_tail
                        nc.vector.tensor_reduce(
                            out=Mx4_sb[:, mi, NTC + j : NTC + j + 1], in_=ps4,
                            axis=AXX, op=ALU_MAX,
                        )

            # ---------------- interleaved pipeline ----------------
            # B order: pure tiles (T_tail..TM-1) then tail tiles (0..T_tail-1)
            bseq = list(range(T_tail, TM)) + list(range(T_tail))

            def need_a(bj):
                t = bseq[bj]
                if t < T_tail:
                    return TM  # tail B needs every slot's g (UsT)
                j = t - T_tail
                return max(bounds[shi_pure[j]], t + 1)

            LEAD = max(max(need_a(j) - j for j in range(TM)) + 1, need_a(0))

            a_next = 0
            b_next = 0
            l3_next = 0
            g_emitted = [False] * S
            ust_emitted = False

            def try_unlock():
                nonlocal ust_emitted
                for s in range(S):
                    if not g_emitted[s] and a_next >= bounds[s]:
                        for t in [d for d in deferred_mx2 if d < bounds[s]]:
                            emit_mx2(t)
                            deferred_mx2.remove(t)
                        emit_g(s)
                        g_emitted[s] = True
                if not ust_emitted and all(g_emitted):
                    emit_UsT()
                    ust_emitted = True

            def b_ready(bj):
                t = bseq[bj]
                if t < T_tail:
                    return ust_emitted
                return g_emitted[shi_pure[t - T_tail]] and a_next > t

            while b_next < TM:
                while a_next < min(TM, b_next + LEAD):
                    emit_A(a_next, fill=(b_next == 0))
                    if b_next == 0:
                        # dummy matmuls keep the HAM clock gate open through
                        # the drain-paced fill
                        for _ in range(3):
                            nc.tensor.matmul(
                                ps_dummy[:, :], warm_src[:, 0:128], warm_src[:, :],
                                start=True, stop=True,
                            )
                    a_next += 1
                    try_unlock()
                progressed = False
                while l3_next <= min(b_next + 1, TM - 1) and b_ready(l3_next):
                    emit_L3(bseq[l3_next])
                    l3_next += 1
                    progressed = True
                if l3_next > b_next:
                    if b_next == 0:
                        # bridge the prologue stall (L4(0) waits on first h3)
                        for _ in range(20):
                            nc.tensor.matmul(
                                ps_dummy[:, :], warm_src[:, 0:128], warm_src[:, :],
                                start=True, stop=True,
                            )
                    # flush one deferred Mx2 per B tile into steady-state slack
                    if deferred_mx2:
                        emit_mx2(deferred_mx2.pop(0))
                    emit_L4(bseq[b_next])
                    b_next += 1
                    progressed = True
                if not progressed:
                    if a_next < TM:
                        emit_A(a_next, fill=(b_next == 0))
                        a_next += 1
                        try_unlock()
                    else:
                        raise RuntimeError("pipeline deadlock")

            nc.sync.dma_start(out=mx4.ap(), in_=Mx4_sb)

    nc.finalize()
    return nc


def _partition(npts: np.ndarray, n_cores: int, slots: int):
    """Assign whole segments to cores, balancing total points."""
    B = len(npts)
    order = np.argsort(-npts, kind="stable")
    best = None
    for trial in range(64):
        rng = np.random.default_rng(trial)
        seq = order.copy() if trial == 0 else rng.permutation(B)
        seq = sorted(seq, key=lambda s: -npts[s])
        if trial > 0:  # tie-break shuffles
            k = trial % 4 + 1
            seq = list(seq)
            for i in range(0, len(seq) - k, k):
                sub = seq[i : i + k]
                rng.shuffle(sub)
                seq[i : i + k] = sub
        groups = [[] for _ in range(n_cores)]
        loads = [0] * n_cores
        for s in seq:
            cands = [c for c in range(n_cores) if len(groups[c]) < slots]
            c = min(cands, key=lambda i: loads[i])
            groups[c].append(int(s))
            loads[c] += int(npts[s])
        for _ in range(400):
            hi = max(range(n_cores), key=lambda i: loads[i])
            done = True
            for lo in sorted(range(n_cores), key=lambda i: loads[i]):
                if lo == hi:
                    continue
                for ia, sa in enumerate(groups[hi]):
                    for ib, sb in enumerate(groups[lo]):
                        d = int(npts[sa]) - int(npts[sb])
                        if d > 0 and max(loads[hi] - d, loads[lo] + d) < loads[hi]:
                            groups[hi][ia], groups[lo][ib] = sb, sa
                            loads[hi] -= d
                            loads[lo] += d
                            done = False
                            break
                    if not done:
                        break
                if not done:
                    break
            if done:
                break
        key = (max(loads), tuple(sorted(loads)))
        if best is None or key < best[0]:
            best = (key, [list(g) for g in groups])
    return best[1]


def _prepare(x: np.ndarray, seg_ids: np.ndarray, B: int):
    counts = np.bincount(seg_ids, minlength=B)
    starts = np.concatenate([[0], np.cumsum(counts)])
    S = (B + N_CORES - 1) // N_CORES

    groups = _partition(counts.astype(np.int64), N_CORES, S)
    # slot 0 smallest so phase B unlocks early
    for c in range(N_CORES):
        groups[c] = sorted(groups[c], key=lambda s: counts[s])

    fulls = [sum(int(counts[s]) // PT for s in g) for g in groups]
    rem64 = [
        sum(-(-(int(counts[s]) % PT) // GRP) * GRP for s in g if int(counts[s]) % PT)
        for g in groups
    ]

    best = None
    for P_pure in range(max(1, min(fulls) - 2), max(fulls) + 1):
        T_tail = 1
        for c in range(N_CORES):
            tail_pts = rem64[c] + max(0, fulls[c] - P_pure) * PT
            T_tail = max(T_tail, -(-tail_pts // PT))
        TM = P_pure + T_tail
        if best is None or (TM, T_tail) < best[:2]:
            best = (TM, T_tail, P_pure)
    TM, T_tail, P_pure = best
    NTC = T_tail * NG
    GW = NTC + P_pure

    xT_cores, gmask_cores, umask2_cores, onehot_cores, post = [], [], [], [], []
    pure_slots_all = []
    for c in range(N_CORES):
        segs = groups[c]
        demote = max(0, fulls[c] - P_pure)
        pure_blocks = []  # (slot, pts[512,3])
        tail_parts = []  # (slot, pts[n*64,3])
        for k, s in enumerate(segs):
            pts = x[starts[s] : starts[s + 1]]
            nf = len(pts) // PT
            rem = pts[nf * PT :]
            if len(rem):
                while len(rem) % GRP:
                    rem = np.concatenate([rem, rem[: GRP - len(rem) % GRP]])
                tail_parts.append((k, rem))
            for d in range(nf):
                pure_blocks.append((k, pts[d * PT : (d + 1) * PT]))
        # demote full blocks (from the end = largest slots) into the tail
        for _ in range(demote):
            k, blk = pure_blocks.pop()
            tail_parts.append((k, blk))
        while len(pure_blocks) < P_pure:
            pure_blocks.append(pure_blocks[-1])
        assert len(pure_blocks) == P_pure, (c, len(pure_blocks), P_pure)
        if not tail_parts:
            k = len(segs) - 1
            tail_parts.append(
                (k, np.tile(x[starts[segs[k]] : starts[segs[k]] + 1], (GRP, 1)))
            )
        tail_pts = np.concatenate([p for _, p in tail_parts])
        tail_grp_slot = sum(([k] * (len(p) // GRP) for k, p in tail_parts), [])
        need = T_tail * PT - len(tail_pts)
        assert need >= 0, (c, len(tail_pts))
        if need:
            lastk = tail_parts[-1][0]
            reps = np.tile(tail_parts[-1][1][:GRP], (need // GRP, 1))
            tail_pts = np.concatenate([tail_pts, reps])
            tail_grp_slot += [lastk] * (need // GRP)
        assert len(tail_grp_slot) == NTC

        pure_slots = [k for k, _ in pure_blocks]
        pure_slots_all.append(pure_slots)

        xc = np.concatenate([tail_pts] + [p for _, p in pure_blocks]).astype(np.float16)
        xT_cores.append(np.ascontiguousarray(xc.T))

        gm = np.zeros((S, GW), np.float32)
        for col, k in enumerate(tail_grp_slot):
            gm[k, col] = 1.0
        for j, k in enumerate(pure_slots):
            gm[k, NTC + j] = 1.0
        gmask_cores.append(np.ascontiguousarray(np.broadcast_to(gm[None], (128, S, GW))))

        um = np.zeros((S, P_pure), np.float32)
        for j, k in enumerate(pure_slots):
            um[k, j] = 1.0
        umask2_cores.append(
            np.ascontiguousarray(np.broadcast_to(um[None], (128, S, P_pure)))
        )

        oh = np.zeros((S, T_tail * PT), np.float16)
        for gcol, k in enumerate(tail_grp_slot):
            oh[k, gcol * GRP : (gcol + 1) * GRP] = 1.0
        onehot_cores.append(oh)

        post.append((segs, pure_slots, tail_grp_slot))

    bounds = tuple(
        T_tail + max(
            (max((j + 1 for j, k in enumerate(psl) if k <= s), default=0))
            for psl in pure_slots_all
        )
        for s in range(S)
    )
    shi_pure = tuple(
        max(pure_slots_all[c][j] for c in range(N_CORES)) for j in range(P_pure)
    )
    return (
        (T_tail, P_pure, S, bounds, shi_pure),
        xT_cores, gmask_cores, umask2_cores, onehot_cores, post,
    )


def make_in_maps(inputs):
    x = np.asarray(inputs["x"], np.float32)
    seg_ids = np.asarray(inputs["seg_ids"])
    B = int(inputs["num_segments"])

    Wf, bf = [], []
    for i in (1, 2, 3, 4):
        W = np.asarray(inputs[f"W{i}"], np.float32)
        b = np.asarray(inputs[f"b{i}"], np.float32)
        ga = np.asarray(inputs[f"g{i}"], np.float32)
        be = np.asarray(inputs[f"be{i}"], np.float32)
        m = np.asarray(inputs[f"m{i}"], np.float32)
        v = np.asarray(inputs[f"v{i}"], np.float32)
        sc = ga / np.sqrt(v + EPS)
        Wf.append(np.ascontiguousarray(W * sc[None, :]))
        bf.append((b - m) * sc + be)
    W1f, W2f, W3f, W4f = Wf
    b1f, b2f, b3f, b4f = bf

    key, xT_cores, gmask_cores, umask2_cores, onehot_cores, post = _prepare(x, seg_ids, B)

    w1d = W1f.astype(np.float16)
    w2d = np.ascontiguousarray(W2f.reshape(128, 2, 128).astype(np.float16))
    w3ad = np.ascontiguousarray(
        W3f[:256].reshape(2, 128, 4, 128).transpose(1, 0, 2, 3).astype(np.float16)
    )
    w3bd = np.ascontiguousarray(
        W3f[256:].reshape(2, 128, 4, 128).transpose(1, 0, 2, 3).astype(np.float16)
    )
    w4d = np.ascontiguousarray(
        W4f.reshape(4, 128, 8, 128).transpose(1, 0, 2, 3).astype(np.float16)
    )
    b1d = np.ascontiguousarray(b1f.reshape(128, 1))
    b2d = np.ascontiguousarray(b2f.reshape(2, 128).T)
    b3d = np.ascontiguousarray(b3f.reshape(4, 128).T)

    in_maps = [
        {
            "xT": xT_cores[c],
            "gmask": gmask_cores[c],
            "umask2": umask2_cores[c],
            "onehot": onehot_cores[c],
            "w1": w1d,
            "w2": w2d,
            "w3a": w3ad,
            "w3b": w3bd,
            "w4": w4d,
            "b1": b1d,
            "b2": b2d,
            "b3": b3d,
        }
        for c in range(N_CORES)
    ]
    return key, in_maps, post, b4f


def postprocess(results, post, b4f, B, T_tail, P_pure):
    NTC = T_tail * NG
    out = np.zeros((B, 1024), np.float32)
    for c in range(N_CORES):
        mx4 = results[c]["mx4"]  # [128, 8, GW]
        segs, pure_slots, tail_grp_slot = post[c]
        for k, s in enumerate(segs):
            cols = [g for g, kk in enumerate(tail_grp_slot) if kk == k]
            cols += [NTC + j for j, kk in enumerate(pure_slots) if kk == k]
            raw = mx4[:, :, cols].max(axis=2)  # [128, 8]
            out[s] = np.maximum(raw.T.reshape(1024) + b4f, 0.0)
    return out


def get_program(key):
    if key not in _PROGRAM_CACHE:
        _PROGRAM_CACHE[key] = _build_program(*key)
    return _PROGRAM_CACHE[key]


def kernel(**inputs) -> np.ndarray:
    B = int(inputs["num_segments"])
    key, in_maps, post, b4f = make_in_maps(inputs)
    nc = get_program(key)
    last_err = None
    for _ in range(3):  # retry transient NRT device wedges
        try:
            res = run_bass_kernel_spmd(nc, in_maps, core_ids=list(range(N_CORES)))
            return postprocess(res.results, post, b4f, B, key[0], key[1])
        except Exception as e:  # noqa: BLE001
            last_err = e
    raise last_err


# revision 19
# speedup vs baseline: 1.0981x; 1.0120x over previous
"""PointNet-style encoder (conv1x1 stack + ragged segment-max) on 8 Trainium2 cores.

Strategy (v2.1 — tail-tile packing)
-----------------------------------
* BN folded into conv weights host-side; every layer is matmul+bias+ReLU.
* Feature-major on device: activations live as [C, points] tiles; points stream
  through the PE as the matmul free dimension in 512-point macro-tiles.
* Segments are point-balanced across the 8 cores (whole segments per core, so
  the two segment-maxes stay core-local). Per core the layout is:
    - T_TAIL fixed "tail" tiles (array idx 0..T_TAIL-1) holding every
      segment's sub-512 remainder, packed as 64-col-aligned per-slot chunks;
    - P_PURE "pure" tiles (idx T_TAIL..) each belonging to one segment.
  This wastes ~1 tile/core instead of the ~3 of per-segment padding.
* Phase A (L1+L2) runs tail tiles first, then pures in slot order; per-tile
  f2 maxes (Mx2) land in a combined column buffer: 8 per-64-col group maxes
  per tail tile, 1 per pure tile. Per-slot g = masked max over that buffer
  (gmask input zeros other slots; buffer memset 0 + f2>=0 keeps unwritten
  columns neutral).
* Mid-network unpool via concat identity: concat(f2, g)@W3 = f2@W3a + g@W3b.
  Per slot Us = W3b^T g + b3 (tiny 1-col matmuls). Pure tiles read a per-tile
  bias column Ub[:,m,j], built by one masked broadcast pass per slot (umask2
  input). Tail tiles get the g-term exactly via an extra accumulated matmul
  per m-chunk: lhsT = UsT (computed on-PE as G2^T@W3b, [S,512]), rhs =
  one-hot slot matrix O [S, cols] — per-point unpool without masks.
* Phase B (L3+L4) runs pures first (unlocked per slot as g arrives), tail
  tiles last (they need every slot's UsT). L4 maxes reduce per pure tile and
  per 64-col group for tail tiles (raw, pre-bias); the host applies
  relu(.+b4) and combines columns per segment (exact under max).
* Single interleaved pipeline (phase A runs LEAD tiles ahead of phase B) so
  A's ACT/DVE drains hide under B's PE-bound matmuls; dummy matmuls keep the
  PE HAM clock-gate open through the drain-paced fill phase.
* Matmuls in float16 (fp32 PSUM): 1 cycle/column, ~1e-3 rel err.
"""

import numpy as np

import concourse.bass as bass
import concourse.mybir as mybir
import concourse.tile as tile
from concourse import bacc
from concourse.bass_utils import run_bass_kernel_spmd

N_CORES = 8
PT = 512
GRP = 64  # tail group granularity (cols)
NG = PT // GRP  # groups per tile
EPS = 1e-3  # keras BatchNormalization default epsilon

F32 = mybir.dt.float32
F16 = mybir.dt.float16
AF = mybir.ActivationFunctionType
AXX = mybir.AxisListType.X
AXXY = mybir.AxisListType.XY
ALU_MAX = mybir.AluOpType.max
ALU_ADD = mybir.AluOpType.add

_PROGRAM_CACHE: dict = {}


def _build_program(T_tail: int, P_pure: int, S: int, bounds: tuple, shi_pure: tuple):
    """One SPMD program for all cores.

    bounds[s]: #A-tiles after which slot s's f2 is complete on every core.
    shi_pure[j]: max (over cores) slot id of pure tile j.
    """
    TM = T_tail + P_pure
    NTC = T_tail * NG  # tail group columns
    GW = NTC + P_pure  # combined max-buffer width

    nc = bacc.Bacc("TRN2")

    xT = nc.dram_tensor("xT", [3, TM * PT], F16, kind="ExternalInput")
    gmask = nc.dram_tensor("gmask", [128, S, GW], F32, kind="ExternalInput")
    umask2 = nc.dram_tensor("umask2", [128, S, P_pure], F32, kind="ExternalInput")
    onehot = nc.dram_tensor("onehot", [S, T_tail * PT], F16, kind="ExternalInput")
    w1 = nc.dram_tensor("w1", [3, 128], F16, kind="ExternalInput")
    w2 = nc.dram_tensor("w2", [128, 2, 128], F16, kind="ExternalInput")
    w3a = nc.dram_tensor("w3a", [128, 2, 4, 128], F16, kind="ExternalInput")
    w3b = nc.dram_tensor("w3b", [128, 2, 4, 128], F16, kind="ExternalInput")
    w4 = nc.dram_tensor("w4", [128, 4, 8, 128], F16, kind="ExternalInput")
    b1 = nc.dram_tensor("b1", [128, 1], F32, kind="ExternalInput")
    b2 = nc.dram_tensor("b2", [128, 2], F32, kind="ExternalInput")
    b3 = nc.dram_tensor("b3", [128, 4], F32, kind="ExternalInput")
    mx4p = nc.dram_tensor("mx4p", [128, 8, P_pure], F32, kind="ExternalOutput")
    mx4t = nc.dram_tensor("mx4t", [128, 8, NTC], F32, kind="ExternalOutput")

    X_HEAD = min(6, TM)  # tiles in the first (prioritized) x DMA

    with tile.TileContext(nc) as tc:
        with (
            tc.tile_pool(name="const", bufs=1) as constp,
            tc.tile_pool(name="h1p", bufs=3) as h1p,
            tc.tile_pool(name="h3p", bufs=3) as h3p,
            tc.tile_pool(name="tmpp", bufs=4) as tmpp,
            tc.tile_pool(name="psA", bufs=4, space="PSUM") as psA,
            tc.tile_pool(name="psB3", bufs=2, space="PSUM") as psB3,
            tc.tile_pool(name="psB4", bufs=2, space="PSUM") as psB4,
        ):
            # x head first on the sync queue so the first L1 unblocks ASAP
            x_sb = constp.tile([3, TM * PT], F16)
            nc.sync.dma_start(out=x_sb[:, : 2 * PT], in_=xT.ap()[:, : 2 * PT])
            nc.sync.dma_start(
                out=x_sb[:, 2 * PT : X_HEAD * PT], in_=xT.ap()[:, 2 * PT : X_HEAD * PT]
            )
            w1_sb = constp.tile([3, 128], F16)
            nc.sync.dma_start(out=w1_sb, in_=w1.ap())
            w2_sb = constp.tile([128, 2, 128], F16)
            nc.sync.dma_start(out=w2_sb, in_=w2.ap())
            b1_sb = constp.tile([128, 1], F32)
            nc.sync.dma_start(out=b1_sb, in_=b1.ap())
            b2_sb = constp.tile([128, 2], F32)
            nc.sync.dma_start(out=b2_sb, in_=b2.ap())
            b3_sb = constp.tile([128, 4], F32)
            nc.sync.dma_start(out=b3_sb, in_=b3.ap())
            nc.sync.dma_start(out=x_sb[:, X_HEAD * PT :], in_=xT.ap()[:, X_HEAD * PT :])
            # phase-B weights + masks on the gpsimd queue, in need order
            w3a_sb = constp.tile([128, 2, 4, 128], F16)
            nc.gpsimd.dma_start(out=w3a_sb, in_=w3a.ap())
            gmask_sb = constp.tile([128, S, GW], F32)
            nc.gpsimd.dma_start(out=gmask_sb, in_=gmask.ap())
            umask2_sb = constp.tile([128, S, P_pure], F32)
            nc.gpsimd.dma_start(out=umask2_sb, in_=umask2.ap())
            w3b_sb = constp.tile([128, 2, 4, 128], F16)
            nc.gpsimd.dma_start(out=w3b_sb, in_=w3b.ap())
            w4_sb = constp.tile([128, 4, 8, 128], F16)
            nc.gpsimd.dma_start(out=w4_sb, in_=w4.ap())
            onehot_sb = constp.tile([S, T_tail * PT], F16)
            nc.gpsimd.dma_start(out=onehot_sb, in_=onehot.ap())

            # f2 storage: tail tiles keep a grouped view for 64-col reduces
            f2_tail = constp.tile([128, T_tail, 2, NG, GRP], F16)
            f2_pure = constp.tile([128, P_pure, 2, PT], F16)
            Mx2_sb = constp.tile([128, 2, GW], F32)
            g_sb = constp.tile([128, 2, S], F32)
            G2s_sb = constp.tile([128, 2, S], F16)
            Us_sb = constp.tile([128, 4, S], F32)
            Ub_sb = constp.tile([128, 4, P_pure], F32)
            UsT_sb = constp.tile([S, 4, 128], F16)
            Mx4p_sb = constp.tile([128, 8, P_pure], F32)
            Mx4t_sb = constp.tile([128, 8, NTC], F32)

            # zero-init buffers that masked ops may read before fully written
            nc.vector.memset(Mx2_sb, 0.0)
            nc.vector.memset(Us_sb, 0.0)
            nc.vector.memset(G2s_sb, 0.0)

            # HAM warmup: dependency-free matmuls during the DMA prologue
            warm_src = constp.tile([128, PT], F16, name="warm_src")
            nc.vector.memset(warm_src, 0.01)
            warm_out = constp.tile([128, 1], F32, name="warm_out")
            warm_act = constp.tile([128, 8], F32, name="warm_act")
            nc.scalar.activation(out=warm_act, in_=warm_src[:, 0:8], func=AF.Relu)
            ps_dummy = psB4.tile([128, PT], F32, tag="ps4", name="ps_warm")
            for _ in range(5):
                nc.tensor.matmul(
                    ps_dummy[:, :], warm_src[:, 0:128], warm_src[:, :], start=True, stop=True
                )
            nc.vector.tensor_reduce(out=warm_out, in_=ps_dummy[:, 0:8], axis=AXX, op=ALU_MAX)

            def f2v(t):
                return f2_tail[:, t] if t < T_tail else f2_pure[:, t - T_tail]

            deferred_mx2: list = []

            def emit_mx2(t):
                if t < T_tail:
                    nc.vector.tensor_reduce(
                        out=Mx2_sb[:, :, t * NG : (t + 1) * NG], in_=f2_tail[:, t],
                        axis=AXX, op=ALU_MAX,
                    )
                else:
                    j = t - T_tail
                    nc.vector.tensor_reduce(
                        out=Mx2_sb[:, :, NTC + j : NTC + j + 1], in_=f2_pure[:, j],
                        axis=AXX, op=ALU_MAX,
                    )

            def dummy_mm(n=1):
                for _ in range(n):
                    nc.tensor.matmul(
                        ps_dummy[:, :], warm_src[:, 0:128], warm_src[:, :],
                        start=True, stop=True,
                    )

            def emit_A(t, fill):
                """L1+L2 for array tile t; stores f2 (fp16) + its Mx2 column(s)."""
                tail = t < T_tail
                shp = [128, NG, GRP] if tail else [128, PT]
                ps1 = psA.tile(shp, F32, tag="psa", name=f"ps1_{t}")
                nc.tensor.matmul(
                    ps1[:, :], w1_sb[:, :], x_sb[:, t * PT : (t + 1) * PT],
                    start=True, stop=True,
                )
                if fill:
                    # dummies fill the h1-ACT latency + keep the HAM gate open
                    dummy_mm(2)
                h1_sb = h1p.tile(shp, F16, tag="h1", name=f"h1_{t}")
                nc.scalar.activation(out=h1_sb, in_=ps1, func=AF.Relu, bias=b1_sb[:, 0:1])
                for c in range(2):
                    ps2 = psA.tile(shp, F32, tag="psa", name=f"ps2_{t}_{c}")
                    nc.tensor.matmul(ps2[:, :], w2_sb[:, c, :], h1_sb[:, :], start=True, stop=True)
                    if fill:
                        dummy_mm(1)
                    # engine split: fill is drain-paced, steady state hides
                    # drains under phase B; Mx2 for pre-bounds[0] tiles must
                    # run inline (slot 0 unlock), later fill tiles defer
                    if fill and t >= bounds[0]:
                        on_dve = True  # both chunks on DVE, no Mx2 inline
                    else:
                        on_dve = c == 1 and fill
                    if not fill:
                        on_dve = False  # steady: ACT takes all three drains
                    if on_dve:
                        nc.vector.tensor_scalar(
                            f2v(t)[:, c], ps2, b2_sb[:, c : c + 1], 0.0, ALU_ADD, ALU_MAX
                        )
                    else:
                        nc.scalar.activation(
                            out=f2v(t)[:, c], in_=ps2, func=AF.Relu, bias=b2_sb[:, c : c + 1]
                        )
                if fill and t >= bounds[0]:
                    deferred_mx2.append(t)
                else:
                    emit_mx2(t)

            def emit_g(s):
                """g[s] = masked max over the Mx2 buffer; then Us[s], Ub pass."""
                for c in range(2):
                    tmp = tmpp.tile([128, GW], F32, tag="tmp", name=f"tmpg_{c}_{s}")
                    nc.vector.tensor_mul(tmp[:, :], Mx2_sb[:, c, :], gmask_sb[:, s, :])
                    nc.vector.tensor_reduce(
                        out=g_sb[:, c, s : s + 1], in_=tmp[:, :], axis=AXX, op=ALU_MAX
                    )
                nc.scalar.copy(G2s_sb[:, :, s], g_sb[:, :, s])
                psu = psA.tile([128, PT], F32, tag="psa", name=f"psu_{s}")
                for m in range(4):
                    nc.tensor.matmul(
                        psu[:, m : m + 1], w3b_sb[:, 0, m, :], G2s_sb[:, 0, s : s + 1],
                        start=True, stop=False,
                    )
                    nc.tensor.matmul(
                        psu[:, m : m + 1], w3b_sb[:, 1, m, :], G2s_sb[:, 1, s : s + 1],
                        start=False, stop=True,
                    )
                for m in range(4):
                    nc.scalar.activation(
                        out=Us_sb[:, m, s : s + 1], in_=psu[:, m : m + 1],
                        func=AF.Identity, bias=b3_sb[:, m : m + 1],
                    )
                # broadcast Us[slot] into the per-tile bias table
                for m in range(4):
                    if s == 0:
                        nc.vector.tensor_scalar_mul(
                            Ub_sb[:, m, :], umask2_sb[:, s, :], Us_sb[:, m, s : s + 1]
                        )
                    else:
                        tmpu = tmpp.tile([128, P_pure], F32, tag="tmpu", name=f"tmpu_{s}_{m}")
                        nc.vector.tensor_scalar_mul(
                            tmpu[:, :], umask2_sb[:, s, :], Us_sb[:, m, s : s + 1]
                        )
                        nc.vector.tensor_add(Ub_sb[:, m, :], Ub_sb[:, m, :], tmpu[:, :])

            def emit_UsT():
                """UsT = (G2s)^T @ W3b  -> [S, 512] fp16 (for tail unpool)."""
                ps = psB3.tile([128, 4, 128], F32, tag="ps3", name="ps_ust")
                nc.tensor.matmul(
                    ps[0:S, :, :], G2s_sb[:, 0, 0:S], w3b_sb[:, 0], start=True, stop=False
                )
                nc.tensor.matmul(
                    ps[0:S, :, :], G2s_sb[:, 1, 0:S], w3b_sb[:, 1], start=False, stop=True
                )
                nc.scalar.copy(UsT_sb[0:S], ps[0:S, :, :])

            h3_tiles = {}

            def emit_L3(t):
                tail = t < T_tail
                h3_sb = h3p.tile([128, 4, PT], F16, tag="h3", name=f"h3_{t}")
                for m in range(4):
                    ps3 = psB3.tile([128, PT], F32, tag="ps3", name=f"ps3_{t}_{m}")
                    nc.tensor.matmul(
                        ps3[:, :], w3a_sb[:, 0, m, :], f2v(t)[:, 0], start=True, stop=False
                    )
                    nc.tensor.matmul(
                        ps3[:, :], w3a_sb[:, 1, m, :], f2v(t)[:, 1],
                        start=False, stop=not tail,
                    )
                    if tail:
                        nc.tensor.matmul(
                            ps3[:, :], UsT_sb[0:S, m, :],
                            onehot_sb[0:S, t * PT : (t + 1) * PT],
                            start=False, stop=True,
                        )
                        bias = b3_sb[:, m : m + 1]
                    else:
                        j = t - T_tail
                        bias = Ub_sb[:, m, j : j + 1]
                    nc.scalar.activation(out=h3_sb[:, m], in_=ps3, func=AF.Relu, bias=bias)
                h3_tiles[t] = h3_sb

            def emit_L4(t):
                tail = t < T_tail
                h3_sb = h3_tiles.pop(t)
                for mi in range(8):
                    shp = [128, NG, GRP] if tail else [128, PT]
                    ps4 = psB4.tile(shp, F32, tag="ps4", name=f"ps4_{t}_{mi}")
                    for k in range(4):
                        nc.tensor.matmul(
                            ps4[:, :], w4_sb[:, k, mi, :], h3_sb[:, k],
                            start=(k == 0), stop=(k == 3),
                        )
                    if tail:
                        nc.vector.tensor_reduce(
                            out=Mx4t_sb[:, mi, t * NG : (t + 1) * NG], in_=ps4,
                            axis=AXX, op=ALU_MAX,
                        )
                    else:
                        j = t - T_tail
                        nc.vector.tensor_reduce(
                            out=Mx4p_sb[:, mi, j : j + 1], in_=ps4,
                            axis=AXX, op=ALU_MAX,
                        )

            # ---------------- interleaved pipeline ----------------
            # B order: pure tiles (T_tail..TM-1) then tail tiles (0..T_tail-1)
            bseq = list(range(T_tail, TM)) + list(range(T_tail))

            def need_a(bj):
                t = bseq[bj]
                if t < T_tail:
                    return TM  # tail B needs every slot's g (UsT)
                j = t - T_tail
                return max(bounds[shi_pure[j]], t + 1)

            LEAD = max(max(need_a(j) - j for j in range(TM)) + 1, need_a(0))

            a_next = 0
            b_next = 0
            l3_next = 0
            g_emitted = [False] * S
            ust_emitted = False

            def try_unlock():
                nonlocal ust_emitted
                for s in range(S):
                    if not g_emitted[s] and a_next >= bounds[s]:
                        for t in [d for d in deferred_mx2 if d < bounds[s]]:
                            emit_mx2(t)
                            deferred_mx2.remove(t)
                        emit_g(s)
                        if s == 0:
                            # cover the g->Us->Ub->L3(0) latency
                            dummy_mm(6)
                        g_emitted[s] = True
                if not ust_emitted and all(g_emitted):
                    emit_UsT()
                    ust_emitted = True

            def b_ready(bj):
                t = bseq[bj]
                if t < T_tail:
                    return ust_emitted
                return g_emitted[shi_pure[t - T_tail]] and a_next > t

            while b_next < TM:
                while a_next < min(TM, b_next + LEAD):
                    emit_A(a_next, fill=(b_next == 0))
                    a_next += 1
                    try_unlock()
                progressed = False
                while l3_next <= min(b_next + 1, TM - 1) and b_ready(l3_next):
                    emit_L3(bseq[l3_next])
                    l3_next += 1
                    progressed = True
                if l3_next > b_next:
                    if b_next == 0:
                        # bridge the prologue stall (L4(0) waits on first h3)
                        dummy_mm(20)
                    # flush one deferred Mx2 per B tile into steady-state slack
                    if deferred_mx2:
                        emit_mx2(deferred_mx2.pop(0))
                    emit_L4(bseq[b_next])
                    b_next += 1
                    progressed = True
                    if b_next == P_pure:
                        # pure columns done; stream them out under the tail work
                        nc.sync.dma_start(out=mx4p.ap(), in_=Mx4p_sb)
                if not progressed:
                    if a_next < TM:
                        emit_A(a_next, fill=(b_next == 0))
                        a_next += 1
                        try_unlock()
                    else:
                        raise RuntimeError("pipeline deadlock")

            nc.sync.dma_start(out=mx4t.ap(), in_=Mx4t_sb)

    nc.finalize()
    return nc


def _partition(npts: np.ndarray, n_cores: int, slots: int):
    """Assign whole segments to cores, balancing total points."""
    B = len(npts)
    order = np.argsort(-npts, kind="stable")
    best = None
    for trial in range(64):
        rng = np.random.default_rng(trial)
        seq = order.copy() if trial == 0 else rng.permutation(B)
        seq = sorted(seq, key=lambda s: -npts[s])
        if trial > 0:  # tie-break shuffles
            k = trial % 4 + 1
            seq = list(seq)
            for i in range(0, len(seq) - k, k):
                sub = seq[i : i + k]
                rng.shuffle(sub)
                seq[i : i + k] = sub
        groups = [[] for _ in range(n_cores)]
        loads = [0] * n_cores
        for s in seq:
            cands = [c for c in range(n_cores) if len(groups[c]) < slots]
            c = min(cands, key=lambda i: loads[i])
            groups[c].append(int(s))
            loads[c] += int(npts[s])
        for _ in range(400):
            hi = max(range(n_cores), key=lambda i: loads[i])
            done = True
            for lo in sorted(range(n_cores), key=lambda i: loads[i]):
                if lo == hi:
                    continue
                for ia, sa in enumerate(groups[hi]):
                    for ib, sb in enumerate(groups[lo]):
                        d = int(npts[sa]) - int(npts[sb])
                        if d > 0 and max(loads[hi] - d, loads[lo] + d) < loads[hi]:
                            groups[hi][ia], groups[lo][ib] = sb, sa
                            loads[hi] -= d
                            loads[lo] += d
                            done = False
                            break
                    if not done:
                        break
                if not done:
                    break
            if done:
                break
        key = (max(loads), tuple(sorted(loads)))
        if best is None or key < best[0]:
            best = (key, [list(g) for g in groups])
    return best[1]


def _prepare(x: np.ndarray, seg_ids: np.ndarray, B: int):
    counts = np.bincount(seg_ids, minlength=B)
    starts = np.concatenate([[0], np.cumsum(counts)])
    S = (B + N_CORES - 1) // N_CORES

    groups = _partition(counts.astype(np.int64), N_CORES, S)
    # slot 0 smallest so phase B unlocks early
    for c in range(N_CORES):
        groups[c] = sorted(groups[c], key=lambda s: counts[s])

    fulls = [sum(int(counts[s]) // PT for s in g) for g in groups]
    rem64 = [
        sum(-(-(int(counts[s]) % PT) // GRP) * GRP for s in g if int(counts[s]) % PT)
        for g in groups
    ]

    best = None
    for P_pure in range(max(1, min(fulls) - 2), max(fulls) + 1):
        T_tail = 1
        for c in range(N_CORES):
            tail_pts = rem64[c] + max(0, fulls[c] - P_pure) * PT
            T_tail = max(T_tail, -(-tail_pts // PT))
        TM = P_pure + T_tail
        if best is None or (TM, T_tail) < best[:2]:
            best = (TM, T_tail, P_pure)
    TM, T_tail, P_pure = best
    NTC = T_tail * NG
    GW = NTC + P_pure

    xT_cores, gmask_cores, umask2_cores, onehot_cores, post = [], [], [], [], []
    pure_slots_all = []
    for c in range(N_CORES):
        segs = groups[c]
        demote = max(0, fulls[c] - P_pure)
        pure_blocks = []  # (slot, pts[512,3])
        tail_parts = []  # (slot, pts[n*64,3])
        for k, s in enumerate(segs):
            pts = x[starts[s] : starts[s + 1]]
            nf = len(pts) // PT
            rem = pts[nf * PT :]
            if len(rem):
                while len(rem) % GRP:
                    rem = np.concatenate([rem, rem[: GRP - len(rem) % GRP]])
                tail_parts.append((k, rem))
            for d in range(nf):
                pure_blocks.append((k, pts[d * PT : (d + 1) * PT]))
        # demote full blocks (from the end = largest slots) into the tail
        for _ in range(demote):
            k, blk = pure_blocks.pop()
            tail_parts.append((k, blk))
        while len(pure_blocks) < P_pure:
            pure_blocks.append(pure_blocks[-1])
        assert len(pure_blocks) == P_pure, (c, len(pure_blocks), P_pure)
        if not tail_parts:
            k = len(segs) - 1
            tail_parts.append(
                (k, np.tile(x[starts[segs[k]] : starts[segs[k]] + 1], (GRP, 1)))
            )
        tail_pts = np.concatenate([p for _, p in tail_parts])
        tail_grp_slot = sum(([k] * (len(p) // GRP) for k, p in tail_parts), [])
        need = T_tail * PT - len(tail_pts)
        assert need >= 0, (c, len(tail_pts))
        if need:
            lastk = tail_parts[-1][0]
            reps = np.tile(tail_parts[-1][1][:GRP], (need // GRP, 1))
            tail_pts = np.concatenate([tail_pts, reps])
            tail_grp_slot += [lastk] * (need // GRP)
        assert len(tail_grp_slot) == NTC

        pure_slots = [k for k, _ in pure_blocks]
        pure_slots_all.append(pure_slots)

        xc = np.concatenate([tail_pts] + [p for _, p in pure_blocks]).astype(np.float16)
        xT_cores.append(np.ascontiguousarray(xc.T))

        gm = np.zeros((S, GW), np.float32)
        for col, k in enumerate(tail_grp_slot):
            gm[k, col] = 1.0
        for j, k in enumerate(pure_slots):
            gm[k, NTC + j] = 1.0
        gmask_cores.append(np.ascontiguousarray(np.broadcast_to(gm[None], (128, S, GW))))

        um = np.zeros((S, P_pure), np.float32)
        for j, k in enumerate(pure_slots):
            um[k, j] = 1.0
        umask2_cores.append(
            np.ascontiguousarray(np.broadcast_to(um[None], (128, S, P_pure)))
        )

        oh = np.zeros((S, T_tail * PT), np.float16)
        for gcol, k in enumerate(tail_grp_slot):
            oh[k, gcol * GRP : (gcol + 1) * GRP] = 1.0
        onehot_cores.append(oh)

        post.append((segs, pure_slots, tail_grp_slot))

    bounds = tuple(
        T_tail + max(
            (max((j + 1 for j, k in enumerate(psl) if k <= s), default=0))
            for psl in pure_slots_all
        )
        for s in range(S)
    )
    shi_pure = tuple(
        max(pure_slots_all[c][j] for c in range(N_CORES)) for j in range(P_pure)
    )
    return (
        (T_tail, P_pure, S, bounds, shi_pure),
        xT_cores, gmask_cores, umask2_cores, onehot_cores, post,
    )


def make_in_maps(inputs):
    x = np.asarray(inputs["x"], np.float32)
    seg_ids = np.asarray(inputs["seg_ids"])
    B = int(inputs["num_segments"])

    Wf, bf = [], []
    for i in (1, 2, 3, 4):
        W = np.asarray(inputs[f"W{i}"], np.float32)
        b = np.asarray(inputs[f"b{i}"], np.float32)
        ga = np.asarray(inputs[f"g{i}"], np.float32)
        be = np.asarray(inputs[f"be{i}"], np.float32)
        m = np.asarray(inputs[f"m{i}"], np.float32)
        v = np.asarray(inputs[f"v{i}"], np.float32)
        sc = ga / np.sqrt(v + EPS)
        Wf.append(np.ascontiguousarray(W * sc[None, :]))
        bf.append((b - m) * sc + be)
    W1f, W2f, W3f, W4f = Wf
    b1f, b2f, b3f, b4f = bf

    key, xT_cores, gmask_cores, umask2_cores, onehot_cores, post = _prepare(x, seg_ids, B)

    w1d = W1f.astype(np.float16)
    w2d = np.ascontiguousarray(W2f.reshape(128, 2, 128).astype(np.float16))
    w3ad = np.ascontiguousarray(
        W3f[:256].reshape(2, 128, 4, 128).transpose(1, 0, 2, 3).astype(np.float16)
    )
    w3bd = np.ascontiguousarray(
        W3f[256:].reshape(2, 128, 4, 128).transpose(1, 0, 2, 3).astype(np.float16)
    )
    w4d = np.ascontiguousarray(
        W4f.reshape(4, 128, 8, 128).transpose(1, 0, 2, 3).astype(np.float16)
    )
    b1d = np.ascontiguousarray(b1f.reshape(128, 1))
    b2d = np.ascontiguousarray(b2f.reshape(2, 128).T)
    b3d = np.ascontiguousarray(b3f.reshape(4, 128).T)

    in_maps = [
        {
            "xT": xT_cores[c],
            "gmask": gmask_cores[c],
            "umask2": umask2_cores[c],
            "onehot": onehot_cores[c],
            "w1": w1d,
            "w2": w2d,
            "w3a": w3ad,
            "w3b": w3bd,
            "w4": w4d,
            "b1": b1d,
            "b2": b2d,
            "b3": b3d,
        }
        for c in range(N_CORES)
    ]
    return key, in_maps, post, b4f


def postprocess(results, post, b4f, B, T_tail, P_pure):
    NTC = T_tail * NG
    out = np.zeros((B, 1024), np.float32)
    for c in range(N_CORES):
        mx4 = np.concatenate([results[c]["mx4t"], results[c]["mx4p"]], axis=2)
        segs, pure_slots, tail_grp_slot = post[c]
        for k, s in enumerate(segs):
            cols = [g for g, kk in enumerate(tail_grp_slot) if kk == k]
            cols += [NTC + j for j, kk in enumerate(pure_slots) if kk == k]
            raw = mx4[:, :, cols].max(axis=2)  # [128, 8]
            out[s] = np.maximum(raw.T.reshape(1024) + b4f, 0.0)
    return out


def get_program(key):
    if key not in _PROGRAM_CACHE:
        _PROGRAM_CACHE[key] = _build_program(*key)
    return _PROGRAM_CACHE[key]


def kernel(**inputs) -> np.ndarray:
    B = int(inputs["num_segments"])
    key, in_maps, post, b4f = make_in_maps(inputs)
    nc = get_program(key)
    last_err = None
    for _ in range(3):  # retry transient NRT device wedges
        try:
            res = run_bass_kernel_spmd(nc, in_maps, core_ids=list(range(N_CORES)))
            return postprocess(res.results, post, b4f, B, key[0], key[1])
        except Exception as e:  # noqa: BLE001
            last_err = e
    raise last_err
